# revision 1
# baseline (speedup 1.0000x reference)
"""Trainium2 Bass kernel for nn_Attention_45749991637079.

Reference computation (per batch b, C=192 channels, 128x128 image, 8 heads):
    qkv  = w_qkv @ x                       (1x1 conv; k-branch unused)
    q,v  = depthwise 3x3 (SAME) of the q/v channel blocks
    qd   = q[:, ::2, ::2]                  (64x64 downsample)
    attn = softmax(l2norm-rows(qd_h) gram * temp)   per head (24x24)
    out  = w_proj @ (attn @ v)             == (w_proj @ blockdiag(attn)) @ v

Sharding: data-parallel over batch; one batch per NeuronCore (8 cores).

v2 design notes (HW-calibrated: per-matmul cost ~ (N + Mcols)/2.1GHz + 25ns,
LDWEIGHTS does not overlap the stream in this toolchain):
  - merged q+v pointwise: one x pass, 3 M-chunks (q0 | v0 | q1+v1 stacked),
    so the two 64-channel tails share full-width matmuls and copies.
  - chunk1 padded buffer pb1 holds q1 on partitions 0:64 and v1 on 64:128;
    q1-tap and v1-tap matmuls sit in disjoint PE quadrants and overlap.
  - v_dw chunk1 and WfT K1 rows live on partitions 64:128 so the final
    matmul needs no cross-partition copies.
  - outputs DMA straight from PSUM to DRAM (no SBUF staging copies).
  - PSUM->SBUF copies split between ACT (q-side) and DVE (v-side).
"""

import numpy as np

C = 192
H = W = 128
HW = H * W
HEADS = 8
CHD = 24
P0, P1 = 128, 64          # channel partition chunks: 0:128 and 128:192
BAND = 16                 # output image rows per band
NB = H // BAND            # 8 bands
PWR = BAND + 2            # pointwise rows computed per band (halo)
PBW = 130                 # padded row width (1 + 128 + 1)
PBSZ = PWR * PBW          # padded band cols per chunk
SUB = 512                 # output subtile cols (4 image rows)
NSUB = BAND * W // SUB    # 4 per band
TAPS = [(di, dj) for di in range(3) for dj in range(3)]
DVE_TAPS = (3, 5)         # v-chunk0 taps computed on the DVE, not the PE

_BUILT = {}


def _build(iters=1):
    import concourse.mybir as mybir
    import concourse.tile as tile
    from concourse import bacc

    f32 = mybir.dt.float32
    f16 = mybir.dt.float16
    Alu = mybir.AluOpType
    Act = mybir.ActivationFunctionType
    Ax = mybir.AxisListType

    nc = bacc.Bacc(
        "TRN2", target_bir_lowering=False, debug=False,
        enable_asserts=False, num_devices=8,
    )

    # DRAM I/O (per-core shapes)
    xb = nc.dram_tensor("xb", (C, HW), f16, kind="ExternalInput").ap()
    wqv = nc.dram_tensor("wqv", (P0, 768), f16, kind="ExternalInput").ap()
    wp = nc.dram_tensor("wp", (P0, 384), f32, kind="ExternalInput").ap()
    dq = nc.dram_tensor("dq", (P0, 9 * P0), f16, kind="ExternalInput").ap()
    dq1 = nc.dram_tensor("dq1", (P1, 9 * P1), f16, kind="ExternalInput").ap()
    dv = nc.dram_tensor("dv", (P0, 9 * P0), f16, kind="ExternalInput").ap()
    dv1b = nc.dram_tensor("dv1b", (P0, 9 * P1), f16, kind="ExternalInput").ap()
    dvw = nc.dram_tensor("dvw", (P0, 9), f32, kind="ExternalInput").ap()
    tq = nc.dram_tensor("tq", (C, 1), f32, kind="ExternalInput").ap()
    eye = nc.dram_tensor("eye", (P0, P0), f16, kind="ExternalInput").ap()
    out = nc.dram_tensor("out", (C, HW), f32, kind="ExternalOutput").ap()
    import os
    _abl = set((os.environ.get("KABL") or "").split(","))  # timing ablations

    import contextlib

    with tile.TileContext(nc) as tc:
      with (tc.For_i(0, iters, 1) if iters > 1 else contextlib.nullcontext()):
        with (
            tc.tile_pool(name="const", bufs=1) as cp,
            tc.tile_pool(name="band", bufs=3) as bp,
            tc.tile_pool(name="work", bufs=3) as wkp,
            tc.tile_pool(name="psA", bufs=4, space="PSUM") as psA,
            tc.tile_pool(name="psH", bufs=2, space="PSUM") as psH,
        ):
            # ---- constants ----
            wqv_sb = cp.tile([P0, 768], f16)
            wp_sb = cp.tile([P0, 384], f32)   # WpT rows 0:128 | rows 128:192
            dq_sb = cp.tile([P0, 9 * P0], f16)
            dq1_sb = cp.tile([P1, 9 * P1], f16)
            dv_sb = cp.tile([P0, 9 * P0], f16)
            dv1b_sb = cp.tile([P0, 9 * P1], f16)  # v1 diags on partitions 64:128
            dvw_sb = cp.tile([P0, 9], f32)        # v0 tap weight columns
            tq_sb = cp.tile([P0, 2], f32)     # [:,0]=ch0..127, [0:64,1]=ch128..191
            eye_sb = cp.tile([P0, P0], f16)
            qd_sb = cp.tile([P0, 8192], f16)  # qd: [:,0:4096] | [0:64,4096:8192]
            vdw_sb = cp.tile([P0, 2 * HW], f16)  # v_dw: [:,0:HW] | [64:128,HW:2HW]
            g0a = cp.tile([P0, C], f32)       # gram accumulator rows 0:128
            g1a = cp.tile([P1, C], f32)       # rows 128:192
            srow = cp.tile([P0, C], f32)      # s_d broadcast to all partitions
            wf_sb = cp.tile([P0, 384], f16)   # WfT K0 | [64:128,192:384] K1
            A0 = cp.tile([P0, C], f32)        # blockdiag(attn) rows 0:128
            A1 = cp.tile([P1, C], f32)        # rows 128:192
            ssq = cp.tile([P0, 2 * NB], f32)  # row sum-of-squares per band
            att = cp.tile([CHD, C], f32)      # per-head attn blocks, compact
            sm8 = cp.tile([CHD, 4 * HEADS], f32)  # softmax stats: max | sum | recip
            rn = cp.tile([P0, 2], f32)        # 1/||q|| * sqrt(temp)
            scr = cp.tile([P0, SUB], f32)     # scratch for sumsq STT

            nc.sync.dma_start(out=wqv_sb[:, 0:640], in_=wqv[:, 0:640])
            nc.sync.dma_start(out=wqv_sb[0:P1, 640:768], in_=wqv[0:P1, 640:768])
            nc.sync.dma_start(out=wp_sb[:, 0:192], in_=wp[:, 0:192])
            nc.sync.dma_start(out=wp_sb[0:P1, 192:384], in_=wp[0:P1, 192:384])
            nc.sync.dma_start(out=dq_sb[:], in_=dq[:])
            nc.sync.dma_start(out=dq1_sb[:], in_=dq1[:])
            nc.sync.dma_start(out=dv_sb[:], in_=dv[:])
            nc.sync.dma_start(out=dv1b_sb[:], in_=dv1b[:])
            nc.sync.dma_start(out=dvw_sb[:], in_=dvw[:])
            nc.sync.dma_start(out=tq_sb[:, 0:1], in_=tq[0:P0, :])
            nc.sync.dma_start(out=tq_sb[0:P1, 1:2], in_=tq[P0:C, :])
            nc.sync.dma_start(out=eye_sb[:], in_=eye[:])

            XBC = PWR * W  # x band cols per chunk (2304)

            nc.gpsimd.memset(g0a[:], 0.0)
            nc.gpsimd.memset(g1a[:], 0.0)

            def dma_xband(b, xband):
                h0 = b * BAND
                r_lo = h0 - 1
                xlo, xhi = max(r_lo, 0), min(r_lo + PWR, H)
                nxc = (xhi - xlo) * W
                nc.sync.dma_start(out=xband[:, 0:nxc],
                                  in_=xb[0:P0, xlo * W:xhi * W])
                nc.sync.dma_start(out=xband[0:P1, XBC:XBC + nxc],
                                  in_=xb[P0:C, xlo * W:xhi * W])

            def pad_pb(b, pbvw):
                nc.gpsimd.memset(pbvw[:, :, 0:1], 0.0)
                nc.gpsimd.memset(pbvw[:, :, 129:130], 0.0)
                if b == 0:
                    nc.gpsimd.memset(pbvw[:, 0, :], 0.0)
                if b == NB - 1:
                    nc.gpsimd.memset(pbvw[:, PWR - 1, :], 0.0)

            def pw_band(b, pbq, pbv, pb1, xband):
                """Merged q+v pointwise conv of band b into 3 padded buffers."""
                r_lo = b * BAND - 1
                xlo = max(r_lo, 0)
                pbqv = pbq[:].rearrange("p (r c) -> p r c", c=PBW)
                pbvv = pbv[:].rearrange("p (r c) -> p r c", c=PBW)
                pb1v = pb1[:].rearrange("p (r c) -> p r c", c=PBW)
                for vw in (pbqv, pbvv, pb1v):
                    pad_pb(b, vw)
                for s in range(6):  # 3-row pw subtiles
                    srw = r_lo + 3 * s
                    v0, v1 = max(srw, 0), min(srw + 3, H)
                    nr = v1 - v0
                    ncols = nr * W
                    xoff = (v0 - xlo) * W
                    lr = v0 - r_lo
                    pq0 = psA.tile([P0, 384], f32, tag="pw",
                                   padded_shape=[P0, SUB])
                    pv0 = psA.tile([P0, 384], f32, tag="pw",
                                   padded_shape=[P0, SUB])
                    p1 = psA.tile([P0, 384], f32, tag="pw",
                                  padded_shape=[P0, SUB])
                    r0 = xband[:, xoff:xoff + ncols]
                    r1 = xband[0:P1, XBC + xoff:XBC + xoff + ncols]
                    if "pw" not in _abl:
                        nc.tensor.matmul(pq0[:, 0:ncols], wqv_sb[:, 0:128], r0,
                                         start=True, stop=False)
                        nc.tensor.matmul(pq0[:, 0:ncols], wqv_sb[0:P1, 128:256],
                                         r1, start=False, stop=True)
                        nc.tensor.matmul(pv0[:, 0:ncols], wqv_sb[:, 256:384], r0,
                                         start=True, stop=False)
                        nc.tensor.matmul(pv0[:, 0:ncols], wqv_sb[0:P1, 384:512],
                                         r1, start=False, stop=True)
                        nc.tensor.matmul(p1[:, 0:ncols], wqv_sb[:, 512:640], r0,
                                         start=True, stop=False)
                        nc.tensor.matmul(p1[:, 0:ncols], wqv_sb[0:P1, 640:768],
                                         r1, start=False, stop=True)
                    else:
                        nc.vector.memset(pq0[:, 0:1], 0.0)
                        nc.vector.memset(pv0[:, 0:1], 0.0)
                        nc.vector.memset(p1[:, 0:1], 0.0)
                    qv0 = pq0[:, 0:ncols].rearrange("p (r c) -> p r c", c=W)
                    vv0 = pv0[:, 0:ncols].rearrange("p (r c) -> p r c", c=W)
                    vv1 = p1[:, 0:ncols].rearrange("p (r c) -> p r c", c=W)
                    nc.scalar.copy(pbqv[:, lr:lr + nr, 1:129], qv0)
                    nc.scalar.copy(pbvv[:, lr:lr + nr, 1:129], vv0)
                    nc.vector.tensor_copy(pb1v[:, lr:lr + nr, 1:129], vv1)
                return pbqv, pbvv, pb1v

            # ========== software-pipelined band sweep ==========
            # stage b: pw(b) | taps(b-1) | gram(b-2).  Every PE op's inputs
            # are produced >=1 stage earlier, so the PE never stalls on the
            # copy engines (and the HAM clock stays warm).
            pb_views = {}

            def pw_stage(b):
                xband = wkp.tile([P0, 2 * XBC], f16, tag="xband")
                dma_xband(b, xband)
                pbq = bp.tile([P0, PBSZ], f16, tag="pbq")
                pbv = bp.tile([P0, PBSZ], f16, tag="pbv")
                pb1 = bp.tile([P0, PBSZ], f16, tag="pb1")
                pb_views[b] = pw_band(b, pbq, pbv, pb1, xband)

            def tap_stage(b):
                h0 = b * BAND
                pbqv, pbvv, pb1v = pb_views.pop(b)
                # q chunk0 taps (full array)
                pqd0 = psH.tile([P0, SUB], f32, tag="tap0")
                pqd1 = psH.tile([P1, SUB], f32, tag="tap1")
                o0 = pqd0[:].rearrange("p (r c) -> p r c", c=64)
                o1 = pqd1[:].rearrange("p (r c) -> p r c", c=64)
                if "qtap" not in _abl:
                    for t, (di, dj) in enumerate(TAPS):
                        st, sp = (t == 0), (t == 8)
                        rhs0 = pbqv[:, di:di + BAND:2, dj:dj + W:2]
                        nc.tensor.matmul(o0, dq_sb[:, t * P0:(t + 1) * P0], rhs0,
                                         start=st, stop=sp)
                else:
                    nc.vector.memset(pqd0[:, 0:1], 0.0)
                # v subtile 0 with q1 taps interleaved: the v1 (64,64) and
                # q1 (0,0) matmuls live in disjoint PE quadrants and overlap.
                ptv0 = psH.tile([P0, SUB], f32, tag="tap0")
                ptv1f = psH.tile([P0, SUB], f32, tag="tap1")
                ptv1 = ptv1f[P1:P0, :]
                ov0 = ptv0[:].rearrange("p (r c) -> p r c", c=W)
                ov1 = ptv1.rearrange("p (r c) -> p r c", c=W)
                if "vtap" not in _abl or "qtap" not in _abl:
                    for t, (di, dj) in enumerate(TAPS):
                        st, sp = (t == 0), (t == 8)
                        if "vtap" not in _abl:
                            if t not in DVE_TAPS:
                                nc.tensor.matmul(
                                    ov0, dv_sb[:, t * P0:(t + 1) * P0],
                                    pbvv[:, di:di + 4, dj:dj + W],
                                    start=st, stop=sp)
                            rhs1 = pb1v[P1:P0, di:di + 4, dj:dj + W]
                            nc.tensor.matmul(ov1,
                                             dv1b_sb[P1:P0, t * P1:(t + 1) * P1],
                                             rhs1, start=st, stop=sp,
                                             tile_position=(P1, P1))
                        if "qtap" not in _abl:
                            rhq1 = pb1v[0:P1, di:di + BAND:2, dj:dj + W:2]
                            nc.tensor.matmul(o1, dq1_sb[:, t * P1:(t + 1) * P1],
                                             rhq1, start=st, stop=sp)
                if "vtap" in _abl:
                    nc.vector.memset(ptv0[:, 0:1], 0.0)
                    nc.vector.memset(ptv1[:, 0:1], 0.0)
                if "qtap" in _abl:
                    nc.vector.memset(pqd1[:, 0:1], 0.0)
                # qd staging (ACT); row sumsq comes from the gram diagonal
                nc.scalar.copy(qd_sb[:, b * SUB:(b + 1) * SUB], pqd0[:])
                nc.scalar.copy(qd_sb[0:P1, 4096 + b * SUB:4096 + (b + 1) * SUB],
                               pqd1[:])
                cs = h0 * W
                nc.vector.tensor_copy(vdw_sb[:, cs:cs + SUB], ptv0[:])
                nc.scalar.copy(vdw_sb[P1:P0, HW + cs:HW + cs + SUB], ptv1)
                # v subtiles 1..3
                for s in range(1, NSUB):
                    ptv0 = psH.tile([P0, SUB], f32, tag="tap0")
                    ptv1f = psH.tile([P0, SUB], f32, tag="tap1")
                    ptv1 = ptv1f[P1:P0, :]
                    ov0 = ptv0[:].rearrange("p (r c) -> p r c", c=W)
                    ov1 = ptv1.rearrange("p (r c) -> p r c", c=W)
                    if "vtap" not in _abl:
                        for t, (di, dj) in enumerate(TAPS):
                            st, sp = (t == 0), (t == 8)
                            rhs0 = pbvv[:, 4 * s + di:4 * s + di + 4, dj:dj + W]
                            rhs1 = pb1v[P1:P0, 4 * s + di:4 * s + di + 4,
                                        dj:dj + W]
                            if t not in DVE_TAPS:
                                nc.tensor.matmul(ov0,
                                                 dv_sb[:, t * P0:(t + 1) * P0],
                                                 rhs0, start=st, stop=sp)
                            nc.tensor.matmul(ov1,
                                             dv1b_sb[P1:P0, t * P1:(t + 1) * P1],
                                             rhs1, start=st, stop=sp,
                                             tile_position=(P1, P1))
                    else:
                        nc.vector.memset(ptv0[:, 0:1], 0.0)
                        nc.vector.memset(ptv1[:, 0:1], 0.0)
                    cs = h0 * W + s * SUB
                    nc.vector.tensor_copy(vdw_sb[:, cs:cs + SUB], ptv0[:])
                    nc.scalar.copy(vdw_sb[P1:P0, HW + cs:HW + cs + SUB], ptv1)
                # DVE-side v0 taps accumulate onto the staged band
                if "vtap" not in _abl:
                    bw = h0 * W
                    av = vdw_sb[:, bw:bw + BAND * W].rearrange(
                        "p (r c) -> p r c", c=W)
                    for t in DVE_TAPS:
                        di, dj = TAPS[t]
                        rhs = pbvv[:, di:di + BAND, dj:dj + W]
                        nc.vector.scalar_tensor_tensor(
                            av, rhs, dvw_sb[:, t:t + 1], av,
                            Alu.mult, Alu.add)

            def gram_stage(b):
                pgb0 = psH.tile([P0, C], f32, tag="tap0")
                pgb1 = psH.tile([P1, C], f32, tag="tap1")
                if "gram" in _abl:
                    nc.vector.memset(pgb0[:, 0:1], 0.0)
                    nc.vector.memset(pgb1[:, 0:1], 0.0)
                for kb in ([] if "gram" in _abl else range(4)):
                    kcol = b * SUB + kb * P0
                    pt0 = psA.tile([P0, P0], f16, tag="pw",
                                   padded_shape=[P0, SUB])
                    pt1 = psA.tile([P0, P1], f16, tag="pw",
                                   padded_shape=[P0, SUB])
                    nc.tensor.transpose(pt0[:], qd_sb[:, kcol:kcol + P0],
                                        eye_sb[:])
                    nc.tensor.transpose(pt1[:],
                                        qd_sb[0:P1, 4096 + kcol:4096 + kcol + P0],
                                        eye_sb[0:P1, 0:P1])
                    qdT = wkp.tile([P0, C], f16, tag="qdT")
                    nc.vector.tensor_copy(qdT[:, 0:P0], pt0[:])
                    nc.vector.tensor_copy(qdT[:, P0:C], pt1[:])
                    nc.tensor.matmul(pgb0[:], qdT[:, 0:P0], qdT[:],
                                     start=(kb == 0), stop=(kb == 3))
                    nc.tensor.matmul(pgb1[:], qdT[:, P0:C], qdT[:],
                                     start=(kb == 0), stop=(kb == 3))
                nc.vector.tensor_tensor(g0a[:], g0a[:], pgb0[:], Alu.add)
                nc.vector.tensor_tensor(g1a[:], g1a[:], pgb1[:], Alu.add)

            for b in range(NB):
                pw_stage(b)
                if b >= 1:
                    tap_stage(b - 1)
                if b >= 2:
                    gram_stage(b - 2)
            tap_stage(NB - 1)
            gram_stage(NB - 2)
            gram_stage(NB - 1)

            # ---- row scales: rn = sqrt(temp) / ||qd_row|| ----
            # ||qd_row||^2 = diag(G): mask G with the identity and row-reduce.
            # ACT Sqrt is low-precision (~4e-3); one Newton step on y=sqrt(ss):
            # y' = 0.5*(y + ss/y), then rn = temp_sqrt / y'.
            nc.vector.tensor_tensor(scr[:, 0:P0], g0a[:, 0:P0], eye_sb[:],
                                    Alu.mult)
            nc.vector.tensor_reduce(ssq[:, 0:1], scr[:, 0:P0], Ax.X, Alu.add)
            nc.vector.tensor_tensor(scr[0:P1, 0:P1], g1a[0:P1, P0:C],
                                    eye_sb[0:P1, 0:P1], Alu.mult)
            nc.vector.tensor_reduce(ssq[0:P1, NB:NB + 1], scr[0:P1, 0:P1],
                                    Ax.X, Alu.add)
            for ss_ap, rn_ap, tq_ap in (
                (ssq[:, 0:1], rn[:, 0:1], tq_sb[:, 0:1]),
                (ssq[0:P1, NB:NB + 1], rn[0:P1, 1:2], tq_sb[0:P1, 1:2]),
            ):
                y = scr[0:ss_ap.shape[0], 0:1]
                yr = scr[0:ss_ap.shape[0], 1:2]
                nc.scalar.activation(y, ss_ap, Act.Sqrt)
                nc.vector.reciprocal(yr, y)                      # 1/y
                nc.vector.tensor_tensor(yr, yr, ss_ap, Alu.mult)  # ss/y
                nc.vector.tensor_tensor(y, y, yr, Alu.add)
                nc.vector.tensor_scalar_mul(y, y, 0.5)            # refined sqrt
                nc.vector.reciprocal(rn_ap, y)
                nc.vector.tensor_tensor(rn_ap, rn_ap, tq_ap, Alu.mult)

            # attn = diag(s) G diag(s): row scale by s_c, then elementwise
            # multiply by s_d replicated across partitions.
            nc.sync.dma_start(out=srow[0:1, 0:P0], in_=rn[:, 0:1])
            nc.sync.dma_start(out=srow[0:1, P0:C], in_=rn[0:P1, 1:2])
            nc.gpsimd.partition_broadcast(srow[:], srow[0:1, :])
            nc.vector.tensor_scalar_mul(g0a[:], g0a[:], rn[:, 0:1])
            nc.vector.tensor_scalar_mul(g1a[:], g1a[:], rn[0:P1, 1:2])
            nc.vector.tensor_tensor(g0a[:], g0a[:], srow[:], Alu.mult)
            nc.vector.tensor_tensor(g1a[:], g1a[:], srow[0:P1, :], Alu.mult)

            # ---- extract per-head diag blocks to compact (24, 8*24) via DMA
            for hh in range(HEADS):
                c0 = hh * CHD
                cs = slice(c0, c0 + CHD)
                dst = att[:, cs]
                if c0 + CHD <= P0:
                    nc.sync.dma_start(out=dst, in_=g0a[cs, cs])
                elif c0 >= P0:
                    nc.sync.dma_start(out=dst, in_=g1a[c0 - P0:c0 - P0 + CHD, cs])
                else:
                    n0 = P0 - c0
                    nc.sync.dma_start(out=att[0:n0, cs], in_=g0a[c0:P0, cs])
                    nc.sync.dma_start(out=att[n0:CHD, cs],
                                      in_=g1a[0:CHD - n0, cs])

            # ---- softmax over d within each head block (compact layout) ----
            attv = att[:].rearrange("p (h c) -> p h c", c=CHD)
            mx = sm8[:, 0:HEADS]
            nc.vector.tensor_reduce(mx, attv, Ax.X, Alu.max)
            nc.vector.tensor_tensor(attv, attv,
                                    mx.unsqueeze(2).broadcast_to((CHD, HEADS, CHD)),
                                    Alu.subtract)
            nc.scalar.activation(att[:], att[:], Act.Exp)
            sm = sm8[:, HEADS:2 * HEADS]
            nc.vector.tensor_reduce(sm, attv, Ax.X, Alu.add)
            rs = sm8[:, 2 * HEADS:3 * HEADS]
            nc.vector.reciprocal(rs, sm)
            nc.vector.tensor_tensor(attv, attv,
                                    rs.unsqueeze(2).broadcast_to((CHD, HEADS, CHD)),
                                    Alu.mult)

            # ---- blockdiag(A) scatter + WfT = A_bd^T-contraction with WpT ----
            nc.gpsimd.memset(A0[:], 0.0)
            nc.gpsimd.memset(A1[:], 0.0)
            for hh in range(HEADS):
                c0 = hh * CHD
                cs = slice(c0, c0 + CHD)
                srcb = att[:, cs]
                if c0 + CHD <= P0:
                    nc.sync.dma_start(out=A0[cs, cs], in_=srcb)
                elif c0 >= P0:
                    nc.sync.dma_start(out=A1[c0 - P0:c0 - P0 + CHD, cs], in_=srcb)
                else:  # head straddling the 128 boundary
                    n0 = P0 - c0
                    nc.sync.dma_start(out=A0[c0:P0, cs], in_=srcb[0:n0, :])
                    nc.sync.dma_start(out=A1[0:CHD - n0, cs], in_=srcb[n0:CHD, :])
            pwf0 = psH.tile([P0, C], f32, tag="tap0")
            pwf1f = psH.tile([P0, C], f32, tag="tap1")
            nc.tensor.matmul(pwf0[:], A0[:, 0:P0], wp_sb[:, 0:192],
                             start=True, stop=False)
            nc.tensor.matmul(pwf0[:], A1[:, 0:P0], wp_sb[0:P1, 192:384],
                             start=False, stop=True)
            nc.tensor.matmul(pwf1f[P1:P0, :], A0[:, P0:C], wp_sb[:, 0:192],
                             start=True, stop=False)
            nc.tensor.matmul(pwf1f[P1:P0, :], A1[:, P0:C], wp_sb[0:P1, 192:384],
                             start=False, stop=True)
            nc.scalar.copy(wf_sb[:, 0:192], pwf0[:])
            nc.scalar.copy(wf_sb[P1:P0, 192:384], pwf1f[P1:P0, :])

            # ========== final sweep: out = WfT-contraction @ v_dw ==========
            for i in range(HW // SUB):
                if i % 2 == 0:
                    po0 = psH.tile([P0, SUB], f32, tag="tap0")
                    po1f = psH.tile([P0, SUB], f32, tag="tap1")
                else:
                    po0 = psA.tile([P0, SUB], f32, tag="pw")
                    po1f = psA.tile([P0, SUB], f32, tag="pw")
                po1 = po1f[0:P1, :]
                r0 = vdw_sb[:, i * SUB:(i + 1) * SUB]
                r1 = vdw_sb[P1:P0, HW + i * SUB:HW + (i + 1) * SUB]
                if "final" not in _abl:
                    nc.tensor.matmul(po0[:], wf_sb[:, 0:P0], r0,
                                     start=True, stop=False)
                    nc.tensor.matmul(po0[:], wf_sb[P1:P0, 192:320], r1,
                                     start=False, stop=True,
                                     tile_position=(P1, 0))
                    nc.tensor.matmul(po1, wf_sb[:, P0:192], r0,
                                     start=True, stop=False)
                    nc.tensor.matmul(po1, wf_sb[P1:P0, 320:384], r1,
                                     start=False, stop=True,
                                     tile_position=(P1, 0))
                else:
                    nc.vector.memset(po0[:, 0:1], 0.0)
                    nc.vector.memset(po1[:, 0:1], 0.0)
                ost0 = wkp.tile([P0, SUB], f32, tag="ost0")
                ost1 = wkp.tile([P1, SUB], f32, tag="ost1")
                nc.scalar.copy(ost0[:], po0[:])
                nc.vector.tensor_copy(ost1[:], po1)
                nc.sync.dma_start(out=out[0:P0, i * SUB:(i + 1) * SUB],
                                  in_=ost0[:])
                nc.sync.dma_start(out=out[P0:C, i * SUB:(i + 1) * SUB],
                                  in_=ost1[:])

    nc.compile()
    return nc


def _host_inputs(x, w_qkv, w_dw, w_proj, temperature):
    """Per-core input maps (host-side precompute of all weight transforms)."""
    f = np.float32
    W_q = w_qkv[0:C].astype(f)           # (192,192) out x in
    W_v = w_qkv[2 * C:3 * C].astype(f)
    wq_d = w_dw[0:C, 0].reshape(C, 9).astype(f)        # (192,9) taps (di,dj)
    wv_d = w_dw[2 * C:3 * C, 0].reshape(C, 9).astype(f)

    WqT = W_q.T.astype(f)                # (in, out)
    WvT = W_v.T.astype(f)
    wqv = np.zeros((P0, 768), f)
    wqv[:, 0:128] = WqT[0:P0, 0:128]
    wqv[0:P1, 128:256] = WqT[P0:C, 0:128]
    wqv[:, 256:384] = WvT[0:P0, 0:128]
    wqv[0:P1, 384:512] = WvT[P0:C, 0:128]
    wqv[:, 512:576] = WqT[0:P0, 128:192]
    wqv[:, 576:640] = WvT[0:P0, 128:192]
    wqv[0:P1, 640:704] = WqT[P0:C, 128:192]
    wqv[0:P1, 704:768] = WvT[P0:C, 128:192]

    def pack_diag(wd, lo, n, base=0, rows=None):
        nr = rows if rows is not None else n
        outm = np.zeros((nr, 9 * n), f)
        for t in range(9):
            np.fill_diagonal(outm[base:base + n, t * n:(t + 1) * n],
                             wd[lo:lo + n, t])
        return outm

    wp_pack = np.zeros((P0, 384), f)
    WpT = w_proj.T.astype(f)
    wp_pack[:, 0:192] = WpT[0:P0]
    wp_pack[0:P1, 192:384] = WpT[P0:C]

    tq = np.sqrt(np.repeat(temperature.reshape(HEADS).astype(f), CHD)).reshape(C, 1)
    eye = np.eye(P0, dtype=np.float16)

    shared = {
        "wqv": wqv, "wp": wp_pack,
        "dq": pack_diag(wq_d, 0, P0),
        "dq1": pack_diag(wq_d, P0, P1),
        "dv": pack_diag(wv_d, 0, P0),
        "dv1b": pack_diag(wv_d, P0, P1, base=P1, rows=P0),
        "dvw": np.ascontiguousarray(wv_d[0:P0]).astype(f),
        "tq": tq, "eye": eye,
    }
    h = np.float16
    for k in ("wqv", "dq", "dq1", "dv", "dv1b"):
        shared[k] = shared[k].astype(h)
    maps = []
    for b in range(8):
        m = dict(shared)
        m["xb"] = np.ascontiguousarray(x[b].reshape(C, HW).astype(h))
        maps.append(m)
    return maps


def kernel(x, w_qkv, w_dw, w_proj, temperature, _trace=False, _iters=1):
    from concourse.bass_utils import run_bass_kernel_spmd
    if _iters not in _BUILT:
        _BUILT[_iters] = _build(_iters)
    nc = _BUILT[_iters]
    in_maps = _host_inputs(
        np.asarray(x), np.asarray(w_qkv), np.asarray(w_dw),
        np.asarray(w_proj), np.asarray(temperature))
    res = run_bass_kernel_spmd(nc, in_maps, list(range(8)), trace=_trace)
    outs = [res.results[i]["out"].reshape(C, H, W) for i in range(8)]
    y = np.stack(outs, axis=0).astype(np.float32)
    kernel.last_result = res
    return y



# revision 2
# speedup vs baseline: 1.3140x; 1.3140x over previous
"""Trainium2 Bass kernel for nn_Attention_45749991637079.

Reference computation (per batch b, C=192 channels, 128x128 image, 8 heads):
    qkv  = w_qkv @ x                       (1x1 conv; k-branch unused)
    q,v  = depthwise 3x3 (SAME) of the q/v channel blocks
    qd   = q[:, ::2, ::2]                  (64x64 downsample)
    attn = softmax(l2norm-rows(qd_h) gram * temp)   per head (24x24)
    out  = w_proj @ (attn @ v)             == (w_proj @ blockdiag(attn)) @ v

Sharding: data-parallel over batch; one batch per NeuronCore (8 cores).

v3 design notes (HW trace: PE busy 82%, middle-section idle 31us, final
sweep down-clocked 2x after the idle):
  - overlap-save pointwise: each image row's 1x1 conv computed exactly once;
    2 halo rows copied from the previous band's padded buffer.
  - chunk1 (64-ch) tap pairing: a flat-shifted duplicate of the chunk1
    plane (one contiguous SBUF->SBUF DMA per band) lets one 128-row
    matmul apply TWO depthwise taps; 9 taps -> 6 streams per subtile.
  - gram transposes issued a band early, decoupled from gram matmuls.
  - v-taps of bands 5..7 deferred until after the softmax chain is
    issued, so the PE chews taps while ACT/DVE/DMA run the middle.
  - masked softmax: per-head max/sum via 0/1 masks, blockdiag(attn) via
    elementwise mask multiply -- no small extract/scatter DMAs.
"""

import numpy as np

C = 192
H = W = 128
HW = H * W
HEADS = 8
CHD = 24
P0, P1 = 128, 64          # channel partition chunks: 0:128 and 128:192
BAND = 16                 # output image rows per band
NB = H // BAND            # 8 bands
PWR = BAND + 2            # padded-buffer rows per band (halo)
PBW = 130                 # padded row width (1 + 128 + 1)
PBSZ = PWR * PBW          # padded band cols per chunk
SUB = 512                 # output subtile cols (4 image rows)
NSUB = BAND * W // SUB    # 4 per band
XBR = 17                  # max x rows loaded per band
XBC = XBR * W
TAPS = [(di, dj) for di in range(3) for dj in range(3)]
DVE_TAPS = (3, 5)         # v-chunk0 taps computed on the DVE, not the PE
DEFER = 5                 # v-taps of bands >= DEFER run after the middle

_BUILT = {}


def _band_rows(b):
    """pb rows [sr, er) computed this band (rest: halo copy / pad)."""
    sr = 1 if b == 0 else 2
    er = 17 if b == NB - 1 else 18
    return sr, er


def _row_chunks(b):
    sr, er = _band_rows(b)
    out = []
    r = sr
    while r < er:
        nr = min(4, er - r)
        out.append((r, nr))
        r += nr
    return out


def _build(iters=1):
    import concourse.mybir as mybir
    import concourse.tile as tile
    from concourse import bacc

    f32 = mybir.dt.float32
    f16 = mybir.dt.float16
    Alu = mybir.AluOpType
    Act = mybir.ActivationFunctionType
    Ax = mybir.AxisListType

    nc = bacc.Bacc(
        "TRN2", target_bir_lowering=False, debug=False,
        enable_asserts=False, num_devices=8,
    )

    # DRAM I/O (per-core shapes)
    xb = nc.dram_tensor("xb", (C, HW), f16, kind="ExternalInput").ap()
    wqv = nc.dram_tensor("wqv", (P0, 768), f16, kind="ExternalInput").ap()
    wp = nc.dram_tensor("wp", (P0, 384), f32, kind="ExternalInput").ap()
    dq = nc.dram_tensor("dq", (P0, 9 * P0), f16, kind="ExternalInput").ap()
    dv = nc.dram_tensor("dv", (P0, 9 * P0), f16, kind="ExternalInput").ap()
    dvw = nc.dram_tensor("dvw", (P0, 9), f32, kind="ExternalInput").ap()
    dv1p = nc.dram_tensor("dv1p", (P0, 3 * P1), f16, kind="ExternalInput").ap()
    dv1s = nc.dram_tensor("dv1s", (P1, 3 * P1), f16, kind="ExternalInput").ap()
    dq1p = nc.dram_tensor("dq1p", (P0, 3 * P1), f16, kind="ExternalInput").ap()
    dq1s = nc.dram_tensor("dq1s", (P0, 3 * P1), f16, kind="ExternalInput").ap()
    tq = nc.dram_tensor("tq", (C, 1), f32, kind="ExternalInput").ap()
    eye = nc.dram_tensor("eye", (P0, P0), f16, kind="ExternalInput").ap()
    bm0 = nc.dram_tensor("bm0", (P0, C), f32, kind="ExternalInput").ap()
    bm1 = nc.dram_tensor("bm1", (P1, C), f32, kind="ExternalInput").ap()
    hm0 = nc.dram_tensor("hm0", (P0, HEADS), f32, kind="ExternalInput").ap()
    hm1 = nc.dram_tensor("hm1", (P1, HEADS), f32, kind="ExternalInput").ap()
    out = nc.dram_tensor("out", (C, HW), f32, kind="ExternalOutput").ap()
    import os
    _abl = set((os.environ.get("KABL") or "").split(","))  # timing ablations

    import contextlib

    with tile.TileContext(nc) as tc:
      with (tc.For_i(0, iters, 1) if iters > 1 else contextlib.nullcontext()):
        with (
            tc.tile_pool(name="const", bufs=1) as cp,
            tc.tile_pool(name="band", bufs=3) as bp,
            tc.tile_pool(name="xb", bufs=2) as xp,
            tc.tile_pool(name="work", bufs=3) as wkp,
            tc.tile_pool(name="qdt", bufs=8) as qp,
            tc.tile_pool(name="psA", bufs=4, space="PSUM") as psA,
            tc.tile_pool(name="psH", bufs=2, space="PSUM") as psH,
        ):
            # ---- constants ----
            wqv_sb = cp.tile([P0, 768], f16)
            wp_sb = cp.tile([P0, 384], f32)   # WpT rows 0:128 | rows 128:192
            dq_sb = cp.tile([P0, 9 * P0], f16)
            dv_sb = cp.tile([P0, 9 * P0], f16)
            dvw_sb = cp.tile([P0, 9], f32)        # v0 tap weight columns
            dv1p_sb = cp.tile([P0, 3 * P1], f16)  # v1 paired taps
            dv1s_sb = cp.tile([P1, 3 * P1], f16)  # v1 single taps (parts 0:64)
            dq1p_sb = cp.tile([P0, 3 * P1], f16)  # q1 paired taps
            dq1s_sb = cp.tile([P0, 3 * P1], f16)  # q1 singles (parts 64:128)
            tq_sb = cp.tile([P0, 2], f32)     # [:,0]=ch0..127, [0:64,1]=ch128..191
            eye_sb = cp.tile([P0, P0], f16)
            bm0_sb = cp.tile([P0, C], f32)    # blockdiag mask rows 0:128
            bm1_sb = cp.tile([P1, C], f32)    # rows 128:192
            hm0_sb = cp.tile([P0, HEADS], f32)  # head-select mask
            hm1_sb = cp.tile([P1, HEADS], f32)
            qd_sb = cp.tile([P0, 8192], f16)  # qd: [:,0:4096] | [0:64,4096:8192]
            vdw_sb = cp.tile([P0, 2 * HW], f16)  # v_dw: [:,0:HW] | [64:128,HW:2HW]
            g0a = cp.tile([P0, C], f32)       # gram accumulator rows 0:128
            g1a = cp.tile([P1, C], f32)       # rows 128:192
            srow = cp.tile([P0, C], f32)      # s_d broadcast to all partitions
            wf_sb = cp.tile([P0, 384], f16)   # WfT K0 | [64:128,192:384] K1
            A0 = cp.tile([P0, C], f32)        # blockdiag(attn) rows 0:128
            A1 = cp.tile([P1, C], f32)        # rows 128:192
            ssq = cp.tile([P0, 2], f32)       # row sum-of-squares
            s8 = cp.tile([P0, 16], f32)       # segment-reduce scratch
            rn = cp.tile([P0, 2], f32)        # 1/||q|| * sqrt(temp)
            scr = cp.tile([P0, SUB], f32)     # scratch

            def load_rest_consts():
                nc.sync.dma_start(out=wp_sb[:, 0:192], in_=wp[:, 0:192])
                nc.sync.dma_start(out=wp_sb[0:P1, 192:384], in_=wp[0:P1, 192:384])
                nc.sync.dma_start(out=dq_sb[:], in_=dq[:])
                nc.sync.dma_start(out=dv_sb[:], in_=dv[:])
                nc.sync.dma_start(out=dvw_sb[:], in_=dvw[:])
                nc.sync.dma_start(out=dv1p_sb[:], in_=dv1p[:])
                nc.sync.dma_start(out=dv1s_sb[:], in_=dv1s[:])
                nc.sync.dma_start(out=dq1p_sb[:], in_=dq1p[:])
                nc.sync.dma_start(out=dq1s_sb[P1:P0, :], in_=dq1s[P1:P0, :])
                nc.sync.dma_start(out=tq_sb[:, 0:1], in_=tq[0:P0, :])
                nc.sync.dma_start(out=tq_sb[0:P1, 1:2], in_=tq[P0:C, :])
                nc.sync.dma_start(out=eye_sb[:], in_=eye[:])
                nc.sync.dma_start(out=bm0_sb[:], in_=bm0[:])
                nc.sync.dma_start(out=bm1_sb[:], in_=bm1[:])
                nc.sync.dma_start(out=hm0_sb[:], in_=hm0[:])
                nc.sync.dma_start(out=hm1_sb[:], in_=hm1[:])
                nc.gpsimd.memset(g0a[:], 0.0)
                nc.gpsimd.memset(g1a[:], 0.0)

            def dma_xband(b, xband):
                sr, er = _band_rows(b)
                xlo = b * BAND - 1 + sr
                xhi = b * BAND - 1 + er
                nxc = (xhi - xlo) * W
                nc.sync.dma_start(out=xband[:, 0:nxc],
                                  in_=xb[0:P0, xlo * W:xhi * W])
                nc.sync.dma_start(out=xband[0:P1, XBC:XBC + nxc],
                                  in_=xb[P0:C, xlo * W:xhi * W])

            # per-band padded buffers, kept across the deferral window
            pb_views = {}   # b -> (pbq, pbv, pbvp, pbqp) flat tiles

            def pw_stage(b):
                xband = xp.tile([P0, 2 * XBC], f16, tag="xband")
                dma_xband(b, xband)
                pbq = bp.tile([P0, PBSZ], f16, tag="pbq")
                pbv = bp.tile([P0, PBSZ], f16, tag="pbv")
                pbvp = bp.tile([P0, PBSZ], f16, tag="pbvp")  # v1: orig 0:64, dup 64:128
                pbqp = bp.tile([P0, PBSZ], f16, tag="pbqp")  # q1: dup 0:64, orig 64:128
                pbqv = pbq[:].rearrange("p (r c) -> p r c", c=PBW)
                pbvv = pbv[:].rearrange("p (r c) -> p r c", c=PBW)
                pvpv = pbvp[:].rearrange("p (r c) -> p r c", c=PBW)
                pqpv = pbqp[:].rearrange("p (r c) -> p r c", c=PBW)
                # side-column pads (left col always; right col where read)
                for vw in (pbqv, pbvv):
                    nc.gpsimd.memset(vw[:, :, 0:1], 0.0)
                    nc.gpsimd.memset(vw[:, :, 129:130], 0.0)
                nc.gpsimd.memset(pvpv[0:P1, :, 0:1], 0.0)
                nc.gpsimd.memset(pvpv[0:P1, :, 129:130], 0.0)
                nc.gpsimd.memset(pqpv[P1:P0, :, 0:1], 0.0)
                nc.gpsimd.memset(pqpv[P1:P0, :, 129:130], 0.0)
                # top/bottom image pad rows
                if b == 0:
                    nc.gpsimd.memset(pbqv[:, 0, :], 0.0)
                    nc.gpsimd.memset(pbvv[:, 0, :], 0.0)
                    nc.gpsimd.memset(pvpv[0:P1, 0, :], 0.0)
                    nc.gpsimd.memset(pqpv[P1:P0, 0, :], 0.0)
                else:
                    # halo: rows 0:2 = previous band's rows 16:18
                    oq, ov, ovp, oqp = pb_views[b - 1]
                    oqv = oq[:].rearrange("p (r c) -> p r c", c=PBW)
                    ovv = ov[:].rearrange("p (r c) -> p r c", c=PBW)
                    ovpv = ovp[:].rearrange("p (r c) -> p r c", c=PBW)
                    oqpv = oqp[:].rearrange("p (r c) -> p r c", c=PBW)
                    nc.scalar.copy(pbqv[:, 0:2, :], oqv[:, 16:18, :])
                    nc.scalar.copy(pbvv[:, 0:2, :], ovv[:, 16:18, :])
                    nc.vector.tensor_copy(pvpv[0:P1, 0:2, :], ovpv[0:P1, 16:18, :])
                    nc.vector.tensor_copy(pqpv[P1:P0, 0:2, :], oqpv[P1:P0, 16:18, :])
                if b == NB - 1:
                    nc.gpsimd.memset(pbqv[:, PWR - 1, :], 0.0)
                    nc.gpsimd.memset(pbvv[:, PWR - 1, :], 0.0)
                    nc.gpsimd.memset(pvpv[0:P1, PWR - 1, :], 0.0)
                    nc.gpsimd.memset(pqpv[P1:P0, PWR - 1, :], 0.0)
                sr, _er = _band_rows(b)
                for rs, nr in _row_chunks(b):
                    ncols = nr * W
                    xoff = (rs - sr) * W
                    pq0 = psA.tile([P0, ncols], f32, tag="pw",
                                   padded_shape=[P0, SUB])
                    pv0 = psA.tile([P0, ncols], f32, tag="pw",
                                   padded_shape=[P0, SUB])
                    p1 = psA.tile([P0, ncols], f32, tag="pw",
                                  padded_shape=[P0, SUB])
                    r0 = xband[:, xoff:xoff + ncols]
                    r1 = xband[0:P1, XBC + xoff:XBC + xoff + ncols]
                    if "pw" not in _abl:
                        nc.tensor.matmul(pq0[:], wqv_sb[:, 0:128], r0,
                                         start=True, stop=False)
                        nc.tensor.matmul(pq0[:], wqv_sb[0:P1, 128:256],
                                         r1, start=False, stop=True)
                        nc.tensor.matmul(pv0[:], wqv_sb[:, 256:384], r0,
                                         start=True, stop=False)
                        nc.tensor.matmul(pv0[:], wqv_sb[0:P1, 384:512],
                                         r1, start=False, stop=True)
                        nc.tensor.matmul(p1[:], wqv_sb[:, 512:640], r0,
                                         start=True, stop=False)
                        nc.tensor.matmul(p1[:], wqv_sb[0:P1, 640:768],
                                         r1, start=False, stop=True)
                    else:
                        nc.vector.memset(pq0[:, 0:1], 0.0)
                        nc.vector.memset(pv0[:, 0:1], 0.0)
                        nc.vector.memset(p1[:, 0:1], 0.0)
                    qv0 = pq0[:].rearrange("p (r c) -> p r c", c=W)
                    vv0 = pv0[:].rearrange("p (r c) -> p r c", c=W)
                    vv1 = p1[:].rearrange("p (r c) -> p r c", c=W)
                    nc.scalar.copy(pbqv[:, rs:rs + nr, 1:129], qv0)
                    nc.scalar.copy(pbvv[:, rs:rs + nr, 1:129], vv0)
                    nc.vector.tensor_copy(pvpv[0:P1, rs:rs + nr, 1:129],
                                          vv1[0:P1])
                    nc.vector.tensor_copy(pqpv[P1:P0, rs:rs + nr, 1:129],
                                          vv1[P1:P0])
                # flat-shifted duplicates: dup[f] = orig[f - 1] so a window
                # at dj reads tap (di, dj-1) on the dup partitions.
                nc.sync.dma_start(out=pbvp[P1:P0, 1:PBSZ],
                                  in_=pbvp[0:P1, 0:PBSZ - 1])
                nc.sync.dma_start(out=pbqp[0:P1, 1:PBSZ],
                                  in_=pbqp[P1:P0, 0:PBSZ - 1])
                pb_views[b] = (pbq, pbv, pbvp, pbqp)

            def qtap_stage(b):
                pbq, _pbv, _pbvp, pbqp = pb_views[b]
                pbqv = pbq[:].rearrange("p (r c) -> p r c", c=PBW)
                pqpv = pbqp[:].rearrange("p (r c) -> p r c", c=PBW)
                pqd0 = psH.tile([P0, SUB], f32, tag="tap0")
                pqd1 = psH.tile([P1, SUB], f32, tag="tap1")
                o0 = pqd0[:].rearrange("p (r c) -> p r c", c=64)
                o1 = pqd1[:].rearrange("p (r c) -> p r c", c=64)
                if "qtap" not in _abl:
                    for t, (di, dj) in enumerate(TAPS):
                        st, sp = (t == 0), (t == 8)
                        rhs0 = pbqv[:, di:di + BAND:2, dj:dj + W:2]
                        nc.tensor.matmul(o0, dq_sb[:, t * P0:(t + 1) * P0], rhs0,
                                         start=st, stop=sp)
                    for di in range(3):
                        # paired: dup parts 0:64 -> tap (di,0); orig -> (di,1)
                        rhp = pqpv[:, di:di + BAND:2, 1:1 + W:2]
                        nc.tensor.matmul(
                            o1, dq1p_sb[:, di * P1:(di + 1) * P1], rhp,
                            start=(di == 0), stop=False)
                        # single: orig parts 64:128 -> tap (di,2)
                        rhs = pqpv[P1:P0, di:di + BAND:2, 2:2 + W:2]
                        nc.tensor.matmul(
                            o1, dq1s_sb[P1:P0, di * P1:(di + 1) * P1], rhs,
                            start=False, stop=(di == 2),
                            tile_position=(P1, 0))
                else:
                    nc.vector.memset(pqd0[:, 0:1], 0.0)
                    nc.vector.memset(pqd1[:, 0:1], 0.0)
                nc.scalar.copy(qd_sb[:, b * SUB:(b + 1) * SUB], pqd0[:])
                nc.scalar.copy(qd_sb[0:P1, 4096 + b * SUB:4096 + (b + 1) * SUB],
                               pqd1[:])

            def vtap_stage(b):
                _pbq, pbv, pbvp, _pbqp = pb_views[b]
                pbvv = pbv[:].rearrange("p (r c) -> p r c", c=PBW)
                pvpv = pbvp[:].rearrange("p (r c) -> p r c", c=PBW)
                h0 = b * BAND
                for s in range(NSUB):
                    ptv0 = psH.tile([P0, SUB], f32, tag="tap0")
                    ptv1f = psH.tile([P0, SUB], f32, tag="tap1")
                    ptv1 = ptv1f[P1:P0, :]
                    ov0 = ptv0[:].rearrange("p (r c) -> p r c", c=W)
                    ov1 = ptv1.rearrange("p (r c) -> p r c", c=W)
                    if "vtap" not in _abl:
                        for t, (di, dj) in enumerate(TAPS):
                            if t in DVE_TAPS:
                                continue
                            st = (t == 0)
                            sp = (t == 8)
                            rhs0 = pbvv[:, 4 * s + di:4 * s + di + 4, dj:dj + W]
                            nc.tensor.matmul(ov0,
                                             dv_sb[:, t * P0:(t + 1) * P0],
                                             rhs0, start=st, stop=sp)
                        for di in range(3):
                            # paired: orig parts 0:64 -> tap (di,1); dup -> (di,0)
                            rhp = pvpv[:, 4 * s + di:4 * s + di + 4, 1:1 + W]
                            nc.tensor.matmul(
                                ov1, dv1p_sb[:, di * P1:(di + 1) * P1], rhp,
                                start=(di == 0), stop=False,
                                tile_position=(0, P1))
                            # single: orig parts 0:64 -> tap (di,2)
                            rhs = pvpv[0:P1, 4 * s + di:4 * s + di + 4, 2:2 + W]
                            nc.tensor.matmul(
                                ov1, dv1s_sb[:, di * P1:(di + 1) * P1], rhs,
                                start=False, stop=(di == 2),
                                tile_position=(0, P1))
                    else:
                        nc.vector.memset(ptv0[:, 0:1], 0.0)
                        nc.vector.memset(ptv1[:, 0:1], 0.0)
                    cs = h0 * W + s * SUB
                    nc.vector.tensor_copy(vdw_sb[:, cs:cs + SUB], ptv0[:])
                    nc.scalar.copy(vdw_sb[P1:P0, HW + cs:HW + cs + SUB], ptv1)
                # DVE-side v0 taps accumulate onto the staged band
                if "vtap" not in _abl:
                    bw = h0 * W
                    av = vdw_sb[:, bw:bw + BAND * W].rearrange(
                        "p (r c) -> p r c", c=W)
                    for t in DVE_TAPS:
                        di, dj = TAPS[t]
                        rhs = pbvv[:, di:di + BAND, dj:dj + W]
                        nc.vector.scalar_tensor_tensor(
                            av, rhs, dvw_sb[:, t:t + 1], av,
                            Alu.mult, Alu.add)

            qdT_tiles = {}

            def trans_stage(b):
                tiles = []
                for kb in range(4):
                    kcol = b * SUB + kb * P0
                    pt0 = psA.tile([P0, P0], f16, tag="pw",
                                   padded_shape=[P0, SUB])
                    pt1 = psA.tile([P0, P1], f16, tag="pw",
                                   padded_shape=[P0, SUB])
                    nc.tensor.transpose(pt0[:], qd_sb[:, kcol:kcol + P0],
                                        eye_sb[:])
                    nc.tensor.transpose(pt1[:],
                                        qd_sb[0:P1, 4096 + kcol:4096 + kcol + P0],
                                        eye_sb[0:P1, 0:P1])
                    qdT = qp.tile([P0, C], f16, tag="qdT")
                    nc.vector.tensor_copy(qdT[:, 0:P0], pt0[:])
                    nc.vector.tensor_copy(qdT[:, P0:C], pt1[:])
                    tiles.append(qdT)
                qdT_tiles[b] = tiles

            def gramm_stage(b):
                pgb0 = psH.tile([P0, C], f32, tag="tap0")
                pgb1 = psH.tile([P1, C], f32, tag="tap1")
                if "gram" in _abl:
                    nc.vector.memset(pgb0[:, 0:1], 0.0)
                    nc.vector.memset(pgb1[:, 0:1], 0.0)
                else:
                    tiles = qdT_tiles.pop(b)
                    for kb in range(4):
                        qdT = tiles[kb]
                        nc.tensor.matmul(pgb0[:], qdT[:, 0:P0], qdT[:],
                                         start=(kb == 0), stop=(kb == 3))
                        nc.tensor.matmul(pgb1[:], qdT[:, P0:C], qdT[:],
                                         start=(kb == 0), stop=(kb == 3))
                nc.vector.tensor_tensor(g0a[:], g0a[:], pgb0[:], Alu.add)
                nc.vector.tensor_tensor(g1a[:], g1a[:], pgb1[:], Alu.add)

            def middle():
                # ---- row scales: rn = sqrt(temp) / ||qd_row|| ----
                # ||qd_row||^2 = diag(G). ACT Sqrt is low-precision (~4e-3);
                # one Newton step on y=sqrt(ss): y' = 0.5*(y + ss/y).
                nc.vector.tensor_tensor(scr[:, 0:P0], g0a[:, 0:P0], eye_sb[:],
                                        Alu.mult)
                nc.vector.tensor_reduce(ssq[:, 0:1], scr[:, 0:P0], Ax.X,
                                        Alu.add)
                nc.vector.tensor_tensor(scr[0:P1, 0:P1], g1a[0:P1, P0:C],
                                        eye_sb[0:P1, 0:P1], Alu.mult)
                nc.vector.tensor_reduce(ssq[0:P1, 1:2], scr[0:P1, 0:P1],
                                        Ax.X, Alu.add)
                for ss_ap, rn_ap, tq_ap in (
                    (ssq[:, 0:1], rn[:, 0:1], tq_sb[:, 0:1]),
                    (ssq[0:P1, 1:2], rn[0:P1, 1:2], tq_sb[0:P1, 1:2]),
                ):
                    y = scr[0:ss_ap.shape[0], 0:1]
                    yr = scr[0:ss_ap.shape[0], 1:2]
                    nc.scalar.activation(y, ss_ap, Act.Sqrt)
                    nc.vector.reciprocal(yr, y)                       # 1/y
                    nc.vector.tensor_tensor(yr, yr, ss_ap, Alu.mult)  # ss/y
                    nc.vector.tensor_tensor(y, y, yr, Alu.add)
                    nc.vector.tensor_scalar_mul(y, y, 0.5)            # sqrt
                    nc.vector.reciprocal(rn_ap, y)
                    nc.vector.tensor_tensor(rn_ap, rn_ap, tq_ap, Alu.mult)

                # logits = diag(s) G diag(s): row scale by s_c, then
                # elementwise multiply by s_d replicated across partitions.
                nc.sync.dma_start(out=srow[0:1, 0:P0], in_=rn[:, 0:1])
                nc.sync.dma_start(out=srow[0:1, P0:C], in_=rn[0:P1, 1:2])
                nc.gpsimd.partition_broadcast(srow[:], srow[0:1, :])
                nc.vector.tensor_scalar_mul(g0a[:], g0a[:], rn[:, 0:1])
                nc.vector.tensor_scalar_mul(g1a[:], g1a[:], rn[0:P1, 1:2])
                nc.vector.tensor_tensor(g0a[:], g0a[:], srow[:], Alu.mult)
                nc.vector.tensor_tensor(g1a[:], g1a[:], srow[0:P1, :],
                                        Alu.mult)

                # ---- masked softmax over the diagonal 24-blocks ----
                for g, hm, bm, A, npd in (
                    (g0a, hm0_sb, bm0_sb, A0, P0),
                    (g1a, hm1_sb, bm1_sb, A1, P1),
                ):
                    gv = g[0:npd, :].rearrange("p (h c) -> p h c", c=CHD)
                    nc.vector.tensor_reduce(s8[0:npd, 0:8], gv, Ax.X, Alu.max)
                    nc.vector.tensor_tensor(s8[0:npd, 8:16], s8[0:npd, 0:8],
                                            hm[0:npd, :], Alu.mult)
                    nc.vector.tensor_reduce(scr[0:npd, 0:1], s8[0:npd, 8:16],
                                            Ax.X, Alu.add)
                    nc.vector.tensor_scalar_sub(g[0:npd, :], g[0:npd, :],
                                                scr[0:npd, 0:1])
                    nc.scalar.activation(g[0:npd, :], g[0:npd, :], Act.Exp)
                    nc.vector.tensor_reduce(s8[0:npd, 0:8], gv, Ax.X, Alu.add)
                    nc.vector.tensor_tensor(s8[0:npd, 8:16], s8[0:npd, 0:8],
                                            hm[0:npd, :], Alu.mult)
                    nc.vector.tensor_reduce(scr[0:npd, 1:2], s8[0:npd, 8:16],
                                            Ax.X, Alu.add)
                    nc.vector.reciprocal(scr[0:npd, 2:3], scr[0:npd, 1:2])
                    nc.vector.scalar_tensor_tensor(
                        A[0:npd, :], g[0:npd, :], scr[0:npd, 2:3],
                        bm[0:npd, :], Alu.mult, Alu.mult)

            def wf_and_final():
                # WfT = (Wp @ blockdiag(attn))^T contraction
                pwf0 = psH.tile([P0, C], f32, tag="tap0")
                pwf1f = psH.tile([P0, C], f32, tag="tap1")
                nc.tensor.matmul(pwf0[:], A0[:, 0:P0], wp_sb[:, 0:192],
                                 start=True, stop=False)
                nc.tensor.matmul(pwf0[:], A1[:, 0:P0], wp_sb[0:P1, 192:384],
                                 start=False, stop=True)
                nc.tensor.matmul(pwf1f[P1:P0, :], A0[:, P0:C], wp_sb[:, 0:192],
                                 start=True, stop=False)
                nc.tensor.matmul(pwf1f[P1:P0, :], A1[:, P0:C],
                                 wp_sb[0:P1, 192:384],
                                 start=False, stop=True)
                nc.scalar.copy(wf_sb[:, 0:192], pwf0[:])
                nc.scalar.copy(wf_sb[P1:P0, 192:384], pwf1f[P1:P0, :])

                # ==== final sweep: out = WfT-contraction @ v_dw ====
                for i in range(HW // SUB):
                    if i % 2 == 0:
                        po0 = psH.tile([P0, SUB], f32, tag="tap0")
                        po1f = psH.tile([P0, SUB], f32, tag="tap1")
                    else:
                        po0 = psA.tile([P0, SUB], f32, tag="pw")
                        po1f = psA.tile([P0, SUB], f32, tag="pw")
                    po1 = po1f[0:P1, :]
                    r0 = vdw_sb[:, i * SUB:(i + 1) * SUB]
                    r1 = vdw_sb[P1:P0, HW + i * SUB:HW + (i + 1) * SUB]
                    if "final" not in _abl:
                        nc.tensor.matmul(po0[:], wf_sb[:, 0:P0], r0,
                                         start=True, stop=False)
                        nc.tensor.matmul(po0[:], wf_sb[P1:P0, 192:320], r1,
                                         start=False, stop=True,
                                         tile_position=(P1, 0))
                        nc.tensor.matmul(po1, wf_sb[:, P0:192], r0,
                                         start=True, stop=False)
                        nc.tensor.matmul(po1, wf_sb[P1:P0, 320:384], r1,
                                         start=False, stop=True,
                                         tile_position=(P1, 0))
                    else:
                        nc.vector.memset(po0[:, 0:1], 0.0)
                        nc.vector.memset(po1[:, 0:1], 0.0)
                    ost0 = wkp.tile([P0, SUB], f32, tag="ost0")
                    ost1 = wkp.tile([P1, SUB], f32, tag="ost1")
                    nc.scalar.copy(ost0[:], po0[:])
                    nc.vector.tensor_copy(ost1[:], po1)
                    nc.sync.dma_start(out=out[0:P0, i * SUB:(i + 1) * SUB],
                                      in_=ost0[:])
                    nc.sync.dma_start(out=out[P0:C, i * SUB:(i + 1) * SUB],
                                      in_=ost1[:])

            # ========== schedule ==========
            nc.sync.dma_start(out=wqv_sb[:, 0:640], in_=wqv[:, 0:640])
            nc.sync.dma_start(out=wqv_sb[0:P1, 640:768], in_=wqv[0:P1, 640:768])
            for b in range(NB):
                pw_stage(b)
                if b == 0:
                    load_rest_consts()
                if b >= 1:
                    qtap_stage(b - 1)
                if b >= 2:
                    gramm_stage(b - 2)
                    if b - 2 < DEFER:
                        vtap_stage(b - 2)
                if b >= 1:
                    trans_stage(b - 1)
            qtap_stage(NB - 1)
            gramm_stage(NB - 2)
            trans_stage(NB - 1)
            vtap_stage(DEFER)
            gramm_stage(NB - 1)
            middle()
            for b in range(DEFER + 1, NB):
                vtap_stage(b)
            wf_and_final()

    nc.compile()
    return nc


def _host_inputs(x, w_qkv, w_dw, w_proj, temperature):
    """Per-core input maps (host-side precompute of all weight transforms)."""
    f = np.float32
    W_q = w_qkv[0:C].astype(f)           # (192,192) out x in
    W_v = w_qkv[2 * C:3 * C].astype(f)
    wq_d = w_dw[0:C, 0].reshape(C, 9).astype(f)        # (192,9) taps (di,dj)
    wv_d = w_dw[2 * C:3 * C, 0].reshape(C, 9).astype(f)

    WqT = W_q.T.astype(f)                # (in, out)
    WvT = W_v.T.astype(f)
    wqv = np.zeros((P0, 768), f)
    wqv[:, 0:128] = WqT[0:P0, 0:128]
    wqv[0:P1, 128:256] = WqT[P0:C, 0:128]
    wqv[:, 256:384] = WvT[0:P0, 0:128]
    wqv[0:P1, 384:512] = WvT[P0:C, 0:128]
    # chunk1 pointwise output order: [v1 (parts 0:64); q1 (parts 64:128)]
    wqv[:, 512:576] = WvT[0:P0, 128:192]
    wqv[:, 576:640] = WqT[0:P0, 128:192]
    wqv[0:P1, 640:704] = WvT[P0:C, 128:192]
    wqv[0:P1, 704:768] = WqT[P0:C, 128:192]

    def pack_diag(wd, lo, n):
        outm = np.zeros((n, 9 * n), f)
        for t in range(9):
            np.fill_diagonal(outm[:, t * n:(t + 1) * n], wd[lo:lo + n, t])
        return outm

    wp_pack = np.zeros((P0, 384), f)
    WpT = w_proj.T.astype(f)
    wp_pack[:, 0:192] = WpT[0:P0]
    wp_pack[0:P1, 192:384] = WpT[P0:C]

    # chunk1 paired/single tap diagonals.
    # v1 buffer: orig at parts 0:64 (window dj=1 -> tap (di,1)),
    #            dup  at parts 64:128 (window dj=1 -> tap (di,0)).
    dv1p_m = np.zeros((P0, 3 * P1), f)
    dv1s_m = np.zeros((P1, 3 * P1), f)
    # q1 buffer: dup at parts 0:64 (-> tap (di,0)), orig at 64:128 (-> (di,1))
    dq1p_m = np.zeros((P0, 3 * P1), f)
    dq1s_m = np.zeros((P0, 3 * P1), f)
    for di in range(3):
        sl = slice(di * P1, (di + 1) * P1)
        np.fill_diagonal(dv1p_m[0:P1, sl], wv_d[P0:C, 3 * di + 1])
        np.fill_diagonal(dv1p_m[P1:P0, sl], wv_d[P0:C, 3 * di + 0])
        np.fill_diagonal(dv1s_m[:, sl], wv_d[P0:C, 3 * di + 2])
        np.fill_diagonal(dq1p_m[0:P1, sl], wq_d[P0:C, 3 * di + 0])
        np.fill_diagonal(dq1p_m[P1:P0, sl], wq_d[P0:C, 3 * di + 1])
        np.fill_diagonal(dq1s_m[P1:P0, sl], wq_d[P0:C, 3 * di + 2])

    tqv = np.sqrt(np.repeat(temperature.reshape(HEADS).astype(f),
                            CHD)).reshape(C, 1)
    eye = np.eye(P0, dtype=np.float16)

    heads0 = np.arange(P0) // CHD
    heads1 = (P0 + np.arange(P1)) // CHD
    headc = np.arange(C) // CHD
    bm0_m = (headc[None, :] == heads0[:, None]).astype(f)
    bm1_m = (headc[None, :] == heads1[:, None]).astype(f)
    hm0_m = (np.arange(HEADS)[None, :] == heads0[:, None]).astype(f)
    hm1_m = (np.arange(HEADS)[None, :] == heads1[:, None]).astype(f)

    shared = {
        "wqv": wqv, "wp": wp_pack,
        "dq": pack_diag(wq_d, 0, P0),
        "dv": pack_diag(wv_d, 0, P0),
        "dvw": np.ascontiguousarray(wv_d[0:P0]).astype(f),
        "dv1p": dv1p_m, "dv1s": dv1s_m, "dq1p": dq1p_m, "dq1s": dq1s_m,
        "tq": tqv, "eye": eye,
        "bm0": bm0_m, "bm1": bm1_m, "hm0": hm0_m, "hm1": hm1_m,
    }
    h = np.float16
    for k in ("wqv", "dq", "dv", "dv1p", "dv1s", "dq1p", "dq1s"):
        shared[k] = shared[k].astype(h)
    maps = []
    for b in range(8):
        m = dict(shared)
        m["xb"] = np.ascontiguousarray(x[b].reshape(C, HW).astype(h))
        maps.append(m)
    return maps


def kernel(x, w_qkv, w_dw, w_proj, temperature, _trace=False, _iters=1):
    from concourse.bass_utils import run_bass_kernel_spmd
    if _iters not in _BUILT:
        _BUILT[_iters] = _build(_iters)
    nc = _BUILT[_iters]
    in_maps = _host_inputs(
        np.asarray(x), np.asarray(w_qkv), np.asarray(w_dw),
        np.asarray(w_proj), np.asarray(temperature))
    res = run_bass_kernel_spmd(nc, in_maps, list(range(8)), trace=_trace)
    outs = [res.results[i]["out"].reshape(C, H, W) for i in range(8)]
    y = np.stack(outs, axis=0).astype(np.float32)
    kernel.last_result = res
    return y


# revision 12
# speedup vs baseline: 1.4319x; 1.0898x over previous
"""Trainium2 Bass kernel for nn_Attention_45749991637079.

Reference computation (per batch b, C=192 channels, 128x128 image, 8 heads):
    qkv  = w_qkv @ x                       (1x1 conv; k-branch unused)
    q,v  = depthwise 3x3 (SAME) of the q/v channel blocks
    qd   = q[:, ::2, ::2]                  (64x64 downsample)
    attn = softmax(l2norm-rows(qd_h) gram * temp)   per head (24x24)
    out  = w_proj @ (attn @ v)             == (w_proj @ blockdiag(attn)) @ v

Sharding: data-parallel over batch; one batch per NeuronCore (8 cores).

v3 design notes (HW trace: PE busy 82%, middle-section idle 31us, final
sweep down-clocked 2x after the idle):
  - overlap-save pointwise: each image row's 1x1 conv computed exactly once;
    2 halo rows copied from the previous band's padded buffer.
  - chunk1 (64-ch) tap pairing: a flat-shifted duplicate of the chunk1
    plane (one contiguous SBUF->SBUF DMA per band) lets one 128-row
    matmul apply TWO depthwise taps; 9 taps -> 6 streams per subtile.
  - gram transposes issued a band early, decoupled from gram matmuls.
  - v-taps of bands 5..7 deferred until after the softmax chain is
    issued, so the PE chews taps while ACT/DVE/DMA run the middle.
  - masked softmax: per-head max/sum via 0/1 masks, blockdiag(attn) via
    elementwise mask multiply -- no small extract/scatter DMAs.
"""

import numpy as np

C = 192
H = W = 128
HW = H * W
HEADS = 8
CHD = 24
P0, P1 = 128, 64          # channel partition chunks: 0:128 and 128:192
BAND = 16                 # output image rows per band
NB = H // BAND            # 8 bands
PWR = BAND + 2            # padded-buffer rows per band (halo)
PBW = 130                 # padded row width (1 + 128 + 1)
PBSZ = PWR * PBW          # padded band cols per chunk
SUB = 512                 # output subtile cols (4 image rows)
NSUB = BAND * W // SUB    # 4 per band
XBR = 17                  # max x rows loaded per band
XBC = XBR * W
TAPS = [(di, dj) for di in range(3) for dj in range(3)]
DVE_TAPS = (3, 5)         # v-chunk0 taps computed on the DVE, not the PE
DEFER = 5                 # v-taps of bands >= DEFER run after the middle

_BUILT = {}


def _band_rows(b):
    """pb rows [sr, er) computed this band (rest: halo copy / pad)."""
    sr = 1 if b == 0 else 2
    er = 17 if b == NB - 1 else 18
    return sr, er


def _row_chunks(b):
    sr, er = _band_rows(b)
    out = []
    r = sr
    while r < er:
        nr = min(4, er - r)
        out.append((r, nr))
        r += nr
    return out


def _build(iters=1):
    import concourse.mybir as mybir
    import concourse.tile as tile
    from concourse import bacc

    f32 = mybir.dt.float32
    f16 = mybir.dt.float16
    Alu = mybir.AluOpType
    Act = mybir.ActivationFunctionType
    Ax = mybir.AxisListType

    nc = bacc.Bacc(
        "TRN2", target_bir_lowering=False, debug=False,
        enable_asserts=False, num_devices=8,
    )

    # DRAM I/O (per-core shapes)
    xb = nc.dram_tensor("xb", (C, HW), f16, kind="ExternalInput").ap()
    wqv = nc.dram_tensor("wqv", (P0, 768), f16, kind="ExternalInput").ap()
    wp = nc.dram_tensor("wp", (P0, 384), f32, kind="ExternalInput").ap()
    dq = nc.dram_tensor("dq", (P0, 9 * P0), f16, kind="ExternalInput").ap()
    dv = nc.dram_tensor("dv", (P0, 9 * P0), f16, kind="ExternalInput").ap()
    dvw = nc.dram_tensor("dvw", (P0, 9), f32, kind="ExternalInput").ap()
    dv1p = nc.dram_tensor("dv1p", (P0, 3 * P1), f16, kind="ExternalInput").ap()
    dv1s = nc.dram_tensor("dv1s", (P1, 3 * P1), f16, kind="ExternalInput").ap()
    dq1p = nc.dram_tensor("dq1p", (P0, 3 * P1), f16, kind="ExternalInput").ap()
    dq1s = nc.dram_tensor("dq1s", (P0, 3 * P1), f16, kind="ExternalInput").ap()
    tq = nc.dram_tensor("tq", (C, 1), f32, kind="ExternalInput").ap()
    eye = nc.dram_tensor("eye", (P0, P0), f16, kind="ExternalInput").ap()
    bm0 = nc.dram_tensor("bm0", (P0, C), f32, kind="ExternalInput").ap()
    bm1 = nc.dram_tensor("bm1", (P1, C), f32, kind="ExternalInput").ap()
    hm0 = nc.dram_tensor("hm0", (P0, HEADS), f32, kind="ExternalInput").ap()
    hm1 = nc.dram_tensor("hm1", (P1, HEADS), f32, kind="ExternalInput").ap()
    out = nc.dram_tensor("out", (C, HW), f16, kind="ExternalOutput").ap()
    import os
    _abl = set((os.environ.get("KABL") or "").split(","))  # timing ablations

    import contextlib

    with tile.TileContext(nc) as tc:
      with (tc.For_i(0, iters, 1) if iters > 1 else contextlib.nullcontext()):
        with (
            tc.tile_pool(name="const", bufs=1) as cp,
            tc.tile_pool(name="band", bufs=3) as bp,
            tc.tile_pool(name="xb", bufs=2) as xp,
            tc.tile_pool(name="work", bufs=3) as wkp,
            tc.tile_pool(name="qdt", bufs=8) as qp,
            tc.tile_pool(name="psA", bufs=4, space="PSUM") as psA,
            tc.tile_pool(name="psH", bufs=2, space="PSUM") as psH,
        ):
            # ---- constants ----
            wqv_sb = cp.tile([P0, 768], f16)
            wp_sb = cp.tile([P0, 384], f32)   # WpT rows 0:128 | rows 128:192
            dq_sb = cp.tile([P0, 9 * P0], f16)
            dv_sb = cp.tile([P0, 9 * P0], f16)
            dvw_sb = cp.tile([P0, 9], f32)        # v0 tap weight columns
            dv1p_sb = cp.tile([P0, 3 * P1], f16)  # v1 paired taps
            dv1s_sb = cp.tile([P1, 3 * P1], f16)  # v1 single taps (parts 0:64)
            dq1p_sb = cp.tile([P0, 3 * P1], f16)  # q1 paired taps
            dq1s_sb = cp.tile([P0, 3 * P1], f16)  # q1 singles (parts 64:128)
            tq_sb = cp.tile([P0, 2], f32)     # [:,0]=ch0..127, [0:64,1]=ch128..191
            eye_sb = cp.tile([P0, P0], f16)
            bm0_sb = cp.tile([P0, C], f32)    # blockdiag mask rows 0:128
            bm1_sb = cp.tile([P1, C], f32)    # rows 128:192
            hm0_sb = cp.tile([P0, HEADS], f32)  # head-select mask
            hm1_sb = cp.tile([P1, HEADS], f32)
            qd_sb = cp.tile([P0, 8192], f16)  # qd: [:,0:4096] | [0:64,4096:8192]
            vdw_sb = cp.tile([P0, 2 * HW], f16)  # v_dw: [:,0:HW] | [64:128,HW:2HW]
            g0a = cp.tile([P0, C], f32)       # gram accumulator rows 0:128
            g1a = cp.tile([P1, C], f32)       # rows 128:192
            srow = cp.tile([P0, C], f32)      # s_d broadcast to all partitions
            wf_sb = cp.tile([P0, 384], f16)   # WfT K0 | [64:128,192:384] K1
            A0 = cp.tile([P0, C], f32)        # blockdiag(attn) rows 0:128
            A1 = cp.tile([P1, C], f32)        # rows 128:192
            ssq = cp.tile([P0, 2], f32)       # row sum-of-squares
            s8 = cp.tile([P0, 16], f32)       # segment-reduce scratch
            rn = cp.tile([P0, 2], f32)        # 1/||q|| * sqrt(temp)
            scr = cp.tile([P0, SUB], f32)     # scratch

            def load_rest_consts():
                nc.sync.dma_start(out=wp_sb[:, 0:192], in_=wp[:, 0:192])
                nc.sync.dma_start(out=wp_sb[0:P1, 192:384], in_=wp[0:P1, 192:384])
                nc.sync.dma_start(out=dq_sb[:], in_=dq[:])
                nc.sync.dma_start(out=dv_sb[:], in_=dv[:])
                nc.sync.dma_start(out=dvw_sb[:], in_=dvw[:])
                nc.sync.dma_start(out=dv1p_sb[:], in_=dv1p[:])
                nc.sync.dma_start(out=dv1s_sb[:], in_=dv1s[:])
                nc.sync.dma_start(out=dq1p_sb[:], in_=dq1p[:])
                nc.sync.dma_start(out=dq1s_sb[P1:P0, :], in_=dq1s[P1:P0, :])
                nc.sync.dma_start(out=tq_sb[:, 0:1], in_=tq[0:P0, :])
                nc.sync.dma_start(out=tq_sb[0:P1, 1:2], in_=tq[P0:C, :])
                nc.sync.dma_start(out=eye_sb[:], in_=eye[:])
                nc.sync.dma_start(out=bm0_sb[:], in_=bm0[:])
                nc.sync.dma_start(out=bm1_sb[:], in_=bm1[:])
                nc.sync.dma_start(out=hm0_sb[:], in_=hm0[:])
                nc.sync.dma_start(out=hm1_sb[:], in_=hm1[:])
                nc.gpsimd.memset(g0a[:], 0.0)
                nc.gpsimd.memset(g1a[:], 0.0)
                # preload the ACT Sqrt/Exp table so the switch doesn't land
                # on the critical band->middle transition
                nc.vector.memset(scr[0:1, 3:4], 1.0)
                nc.scalar.activation(scr[0:1, 4:5], scr[0:1, 3:4], Act.Sqrt)

            def dma_xband(b, xband):
                sr, er = _band_rows(b)
                xlo = b * BAND - 1 + sr
                xhi = b * BAND - 1 + er
                nxc = (xhi - xlo) * W
                nc.sync.dma_start(out=xband[:, 0:nxc],
                                  in_=xb[0:P0, xlo * W:xhi * W])
                nc.sync.dma_start(out=xband[0:P1, XBC:XBC + nxc],
                                  in_=xb[P0:C, xlo * W:xhi * W])

            # per-band padded buffers, kept across the deferral window
            pb_views = {}   # b -> (pbq, pbv, pbvp, pbqp) flat tiles

            def pw_stage(b):
                xband = xp.tile([P0, 2 * XBC], f16, tag="xband")
                dma_xband(b, xband)
                pbq = bp.tile([P0, PBSZ], f16, tag="pbq")
                pbv = bp.tile([P0, PBSZ], f16, tag="pbv")
                pbvp = bp.tile([P0, PBSZ], f16, tag="pbvp")  # v1: orig 0:64, dup 64:128
                pbqp = bp.tile([P0, PBSZ], f16, tag="pbqp")  # q1: dup 0:64, orig 64:128
                pbqv = pbq[:].rearrange("p (r c) -> p r c", c=PBW)
                pbvv = pbv[:].rearrange("p (r c) -> p r c", c=PBW)
                pvpv = pbvp[:].rearrange("p (r c) -> p r c", c=PBW)
                pqpv = pbqp[:].rearrange("p (r c) -> p r c", c=PBW)
                # side-column pads (left col always; right col where read)
                for vw in (pbqv, pbvv):
                    nc.gpsimd.memset(vw[:, :, 0:1], 0.0)
                    nc.gpsimd.memset(vw[:, :, 129:130], 0.0)
                nc.gpsimd.memset(pvpv[0:P1, :, 0:1], 0.0)
                nc.gpsimd.memset(pvpv[0:P1, :, 129:130], 0.0)
                nc.gpsimd.memset(pqpv[P1:P0, :, 0:1], 0.0)
                nc.gpsimd.memset(pqpv[P1:P0, :, 129:130], 0.0)
                # top/bottom image pad rows
                if b == 0:
                    nc.gpsimd.memset(pbqv[:, 0, :], 0.0)
                    nc.gpsimd.memset(pbvv[:, 0, :], 0.0)
                    nc.gpsimd.memset(pvpv[0:P1, 0, :], 0.0)
                    nc.gpsimd.memset(pqpv[P1:P0, 0, :], 0.0)
                else:
                    # halo: rows 0:2 = previous band's rows 16:18
                    oq, ov, ovp, oqp = pb_views[b - 1]
                    oqv = oq[:].rearrange("p (r c) -> p r c", c=PBW)
                    ovv = ov[:].rearrange("p (r c) -> p r c", c=PBW)
                    ovpv = ovp[:].rearrange("p (r c) -> p r c", c=PBW)
                    oqpv = oqp[:].rearrange("p (r c) -> p r c", c=PBW)
                    nc.scalar.copy(pbqv[:, 0:2, :], oqv[:, 16:18, :])
                    nc.scalar.copy(pbvv[:, 0:2, :], ovv[:, 16:18, :])
                    nc.vector.tensor_copy(pvpv[0:P1, 0:2, :], ovpv[0:P1, 16:18, :])
                    nc.vector.tensor_copy(pqpv[P1:P0, 0:2, :], oqpv[P1:P0, 16:18, :])
                if b == NB - 1:
                    nc.gpsimd.memset(pbqv[:, PWR - 1, :], 0.0)
                    nc.gpsimd.memset(pbvv[:, PWR - 1, :], 0.0)
                    nc.gpsimd.memset(pvpv[0:P1, PWR - 1, :], 0.0)
                    nc.gpsimd.memset(pqpv[P1:P0, PWR - 1, :], 0.0)
                sr, _er = _band_rows(b)
                for rs, nr in _row_chunks(b):
                    ncols = nr * W
                    xoff = (rs - sr) * W
                    pq0 = psA.tile([P0, ncols], f32, tag="pw",
                                   padded_shape=[P0, SUB])
                    pv0 = psA.tile([P0, ncols], f32, tag="pw",
                                   padded_shape=[P0, SUB])
                    p1 = psA.tile([P0, ncols], f32, tag="pw",
                                  padded_shape=[P0, SUB])
                    r0 = xband[:, xoff:xoff + ncols]
                    r1 = xband[0:P1, XBC + xoff:XBC + xoff + ncols]
                    if "pw" not in _abl:
                        nc.tensor.matmul(pq0[:], wqv_sb[:, 0:128], r0,
                                         start=True, stop=False)
                        nc.tensor.matmul(pq0[:], wqv_sb[0:P1, 128:256],
                                         r1, start=False, stop=True)
                        nc.tensor.matmul(pv0[:], wqv_sb[:, 256:384], r0,
                                         start=True, stop=False)
                        nc.tensor.matmul(pv0[:], wqv_sb[0:P1, 384:512],
                                         r1, start=False, stop=True)
                        nc.tensor.matmul(p1[:], wqv_sb[:, 512:640], r0,
                                         start=True, stop=False)
                        nc.tensor.matmul(p1[:], wqv_sb[0:P1, 640:768],
                                         r1, start=False, stop=True)
                    else:
                        nc.vector.memset(pq0[:, 0:1], 0.0)
                        nc.vector.memset(pv0[:, 0:1], 0.0)
                        nc.vector.memset(p1[:, 0:1], 0.0)
                    qv0 = pq0[:].rearrange("p (r c) -> p r c", c=W)
                    vv0 = pv0[:].rearrange("p (r c) -> p r c", c=W)
                    vv1 = p1[:].rearrange("p (r c) -> p r c", c=W)
                    nc.scalar.copy(pbqv[:, rs:rs + nr, 1:129], qv0)
                    nc.scalar.copy(pbvv[:, rs:rs + nr, 1:129], vv0)
                    nc.vector.tensor_copy(pvpv[0:P1, rs:rs + nr, 1:129],
                                          vv1[0:P1])
                    nc.vector.tensor_copy(pqpv[P1:P0, rs:rs + nr, 1:129],
                                          vv1[P1:P0])
                # flat-shifted duplicates: dup[f] = orig[f - 1] so a window
                # at dj reads tap (di, dj-1) on the dup partitions.
                nc.sync.dma_start(out=pbvp[P1:P0, 1:PBSZ],
                                  in_=pbvp[0:P1, 0:PBSZ - 1])
                nc.sync.dma_start(out=pbqp[0:P1, 1:PBSZ],
                                  in_=pbqp[P1:P0, 0:PBSZ - 1])
                pb_views[b] = (pbq, pbv, pbvp, pbqp)

            def qtap_stage(b):
                pbq, _pbv, _pbvp, pbqp = pb_views[b]
                pbqv = pbq[:].rearrange("p (r c) -> p r c", c=PBW)
                pqpv = pbqp[:].rearrange("p (r c) -> p r c", c=PBW)
                pqd0 = psH.tile([P0, SUB], f32, tag="tap0")
                pqd1 = psH.tile([P1, SUB], f32, tag="tap1")
                o0 = pqd0[:].rearrange("p (r c) -> p r c", c=64)
                o1 = pqd1[:].rearrange("p (r c) -> p r c", c=64)
                if "qtap" not in _abl:
                    for t, (di, dj) in enumerate(TAPS):
                        st, sp = (t == 0), (t == 8)
                        rhs0 = pbqv[:, di:di + BAND:2, dj:dj + W:2]
                        nc.tensor.matmul(o0, dq_sb[:, t * P0:(t + 1) * P0], rhs0,
                                         start=st, stop=sp)
                    for di in range(3):
                        # paired: dup parts 0:64 -> tap (di,0); orig -> (di,1)
                        rhp = pqpv[:, di:di + BAND:2, 1:1 + W:2]
                        nc.tensor.matmul(
                            o1, dq1p_sb[:, di * P1:(di + 1) * P1], rhp,
                            start=(di == 0), stop=False)
                        # single: orig parts 64:128 -> tap (di,2)
                        rhs = pqpv[P1:P0, di:di + BAND:2, 2:2 + W:2]
                        nc.tensor.matmul(
                            o1, dq1s_sb[P1:P0, di * P1:(di + 1) * P1], rhs,
                            start=False, stop=(di == 2),
                            tile_position=(P1, 0))
                else:
                    nc.vector.memset(pqd0[:, 0:1], 0.0)
                    nc.vector.memset(pqd1[:, 0:1], 0.0)
                nc.scalar.copy(qd_sb[:, b * SUB:(b + 1) * SUB], pqd0[:])
                nc.scalar.copy(qd_sb[0:P1, 4096 + b * SUB:4096 + (b + 1) * SUB],
                               pqd1[:])

            def vtap_stage(b, pe_all=False):
                _pbq, pbv, pbvp, _pbqp = pb_views[b]
                pbvv = pbv[:].rearrange("p (r c) -> p r c", c=PBW)
                pvpv = pbvp[:].rearrange("p (r c) -> p r c", c=PBW)
                h0 = b * BAND
                for s in range(NSUB):
                    if s % 2 == 0:
                        ptv0 = psH.tile([P0, SUB], f32, tag="tap0")
                        ptv1f = psH.tile([P0, SUB], f32, tag="tap1")
                    else:
                        ptv0 = psA.tile([P0, SUB], f32, tag="pw")
                        ptv1f = psA.tile([P0, SUB], f32, tag="pw")
                    ptv1 = ptv1f[P1:P0, :]
                    ov0 = ptv0[:].rearrange("p (r c) -> p r c", c=W)
                    ov1 = ptv1.rearrange("p (r c) -> p r c", c=W)
                    if "vtap" not in _abl:
                        for t, (di, dj) in enumerate(TAPS):
                            if t in DVE_TAPS and not pe_all:
                                continue
                            st = (t == 0)
                            sp = (t == 8)
                            rhs0 = pbvv[:, 4 * s + di:4 * s + di + 4, dj:dj + W]
                            nc.tensor.matmul(ov0,
                                             dv_sb[:, t * P0:(t + 1) * P0],
                                             rhs0, start=st, stop=sp)
                        for di in range(3):
                            # paired: orig parts 0:64 -> tap (di,1); dup -> (di,0)
                            rhp = pvpv[:, 4 * s + di:4 * s + di + 4, 1:1 + W]
                            nc.tensor.matmul(
                                ov1, dv1p_sb[:, di * P1:(di + 1) * P1], rhp,
                                start=(di == 0), stop=False,
                                tile_position=(0, P1))
                            # single: orig parts 0:64 -> tap (di,2)
                            rhs = pvpv[0:P1, 4 * s + di:4 * s + di + 4, 2:2 + W]
                            nc.tensor.matmul(
                                ov1, dv1s_sb[:, di * P1:(di + 1) * P1], rhs,
                                start=False, stop=(di == 2),
                                tile_position=(0, P1))
                    else:
                        nc.vector.memset(ptv0[:, 0:1], 0.0)
                        nc.vector.memset(ptv1[:, 0:1], 0.0)
                    cs = h0 * W + s * SUB
                    nc.vector.tensor_copy(vdw_sb[:, cs:cs + SUB], ptv0[:])
                    nc.scalar.copy(vdw_sb[P1:P0, HW + cs:HW + cs + SUB], ptv1)
                # DVE-side v0 taps accumulate onto the staged band
                if "vtap" not in _abl and not pe_all:
                    bw = h0 * W
                    av = vdw_sb[:, bw:bw + BAND * W].rearrange(
                        "p (r c) -> p r c", c=W)
                    for t in DVE_TAPS:
                        di, dj = TAPS[t]
                        rhs = pbvv[:, di:di + BAND, dj:dj + W]
                        nc.vector.scalar_tensor_tensor(
                            av, rhs, dvw_sb[:, t:t + 1], av,
                            Alu.mult, Alu.add)

            qdT_tiles = {}

            def trans_stage(b):
                tiles = []
                for kb in range(4):
                    kcol = b * SUB + kb * P0
                    pt0 = psA.tile([P0, P0], f16, tag="pw",
                                   padded_shape=[P0, SUB])
                    pt1 = psA.tile([P0, P1], f16, tag="pw",
                                   padded_shape=[P0, SUB])
                    nc.tensor.transpose(pt0[:], qd_sb[:, kcol:kcol + P0],
                                        eye_sb[:])
                    nc.tensor.transpose(pt1[:],
                                        qd_sb[0:P1, 4096 + kcol:4096 + kcol + P0],
                                        eye_sb[0:P1, 0:P1])
                    qdT = qp.tile([P0, C], f16, tag="qdT")
                    nc.vector.tensor_copy(qdT[:, 0:P0], pt0[:])
                    nc.vector.tensor_copy(qdT[:, P0:C], pt1[:])
                    tiles.append(qdT)
                qdT_tiles[b] = tiles

            def gramm_stage(b):
                pgb0 = psH.tile([P0, C], f32, tag="tap0")
                pgb1 = psH.tile([P1, C], f32, tag="tap1")
                if "gram" in _abl:
                    nc.vector.memset(pgb0[:, 0:1], 0.0)
                    nc.vector.memset(pgb1[:, 0:1], 0.0)
                else:
                    tiles = qdT_tiles.pop(b)
                    for kb in range(4):
                        qdT = tiles[kb]
                        nc.tensor.matmul(pgb0[:], qdT[:, 0:P0], qdT[:],
                                         start=(kb == 0), stop=(kb == 3))
                        nc.tensor.matmul(pgb1[:], qdT[:, P0:C], qdT[:],
                                         start=(kb == 0), stop=(kb == 3))
                nc.vector.tensor_tensor(g0a[:], g0a[:], pgb0[:], Alu.add)
                nc.vector.tensor_tensor(g1a[:], g1a[:], pgb1[:], Alu.add)

            def middle():
                # ---- row scales: rn = sqrt(temp) / ||qd_row|| ----
                # ||qd_row||^2 = diag(G). ACT Sqrt is low-precision (~4e-3);
                # one Newton step on y=sqrt(ss): y' = 0.5*(y + ss/y).
                nc.vector.tensor_tensor(scr[:, 0:P0], g0a[:, 0:P0], eye_sb[:],
                                        Alu.mult)
                nc.vector.tensor_reduce(ssq[:, 0:1], scr[:, 0:P0], Ax.X,
                                        Alu.add)
                nc.vector.tensor_tensor(scr[0:P1, 0:P1], g1a[0:P1, P0:C],
                                        eye_sb[0:P1, 0:P1], Alu.mult)
                nc.vector.tensor_reduce(ssq[0:P1, 1:2], scr[0:P1, 0:P1],
                                        Ax.X, Alu.add)
                for ss_ap, rn_ap, tq_ap in (
                    (ssq[:, 0:1], rn[:, 0:1], tq_sb[:, 0:1]),
                    (ssq[0:P1, 1:2], rn[0:P1, 1:2], tq_sb[0:P1, 1:2]),
                ):
                    y = scr[0:ss_ap.shape[0], 0:1]
                    yr = scr[0:ss_ap.shape[0], 1:2]
                    nc.scalar.activation(y, ss_ap, Act.Sqrt)
                    nc.vector.reciprocal(yr, y)                       # 1/y
                    nc.vector.tensor_tensor(yr, yr, ss_ap, Alu.mult)  # ss/y
                    nc.vector.tensor_tensor(y, y, yr, Alu.add)
                    nc.vector.tensor_scalar_mul(y, y, 0.5)            # sqrt
                    nc.vector.reciprocal(rn_ap, y)
                    nc.vector.tensor_tensor(rn_ap, rn_ap, tq_ap, Alu.mult)

                # logits = diag(s) G diag(s): row scale by s_c, then
                # elementwise multiply by s_d replicated across partitions.
                nc.sync.dma_start(out=srow[0:1, 0:P0], in_=rn[:, 0:1])
                nc.sync.dma_start(out=srow[0:1, P0:C], in_=rn[0:P1, 1:2])
                nc.gpsimd.partition_broadcast(srow[:], srow[0:1, :])
                nc.vector.tensor_scalar_mul(g0a[:], g0a[:], rn[:, 0:1])
                nc.vector.tensor_scalar_mul(g1a[:], g1a[:], rn[0:P1, 1:2])
                nc.vector.tensor_tensor(g0a[:], g0a[:], srow[:], Alu.mult)
                nc.vector.tensor_tensor(g1a[:], g1a[:], srow[0:P1, :],
                                        Alu.mult)

                # ---- masked softmax over the diagonal 24-blocks ----
                for g, hm, bm, A, npd in (
                    (g0a, hm0_sb, bm0_sb, A0, P0),
                    (g1a, hm1_sb, bm1_sb, A1, P1),
                ):
                    gv = g[0:npd, :].rearrange("p (h c) -> p h c", c=CHD)
                    nc.vector.tensor_reduce(s8[0:npd, 0:8], gv, Ax.X, Alu.max)
                    nc.vector.tensor_tensor(s8[0:npd, 8:16], s8[0:npd, 0:8],
                                            hm[0:npd, :], Alu.mult)
                    nc.vector.tensor_reduce(scr[0:npd, 0:1], s8[0:npd, 8:16],
                                            Ax.X, Alu.add)
                    nc.vector.tensor_scalar_sub(g[0:npd, :], g[0:npd, :],
                                                scr[0:npd, 0:1])
                    nc.scalar.activation(g[0:npd, :], g[0:npd, :], Act.Exp)
                    nc.vector.tensor_reduce(s8[0:npd, 0:8], gv, Ax.X, Alu.add)
                    nc.vector.tensor_tensor(s8[0:npd, 8:16], s8[0:npd, 0:8],
                                            hm[0:npd, :], Alu.mult)
                    nc.vector.tensor_reduce(scr[0:npd, 1:2], s8[0:npd, 8:16],
                                            Ax.X, Alu.add)
                    nc.vector.reciprocal(scr[0:npd, 2:3], scr[0:npd, 1:2])
                    nc.vector.scalar_tensor_tensor(
                        A[0:npd, :], g[0:npd, :], scr[0:npd, 2:3],
                        bm[0:npd, :], Alu.mult, Alu.mult)

            def wf_and_final():
                # WfT = (Wp @ blockdiag(attn))^T contraction
                pwf0 = psH.tile([P0, C], f32, tag="tap0")
                pwf1f = psH.tile([P0, C], f32, tag="tap1")
                nc.tensor.matmul(pwf0[:], A0[:, 0:P0], wp_sb[:, 0:192],
                                 start=True, stop=False)
                nc.tensor.matmul(pwf0[:], A1[:, 0:P0], wp_sb[0:P1, 192:384],
                                 start=False, stop=True)
                nc.tensor.matmul(pwf1f[P1:P0, :], A0[:, P0:C], wp_sb[:, 0:192],
                                 start=True, stop=False)
                nc.tensor.matmul(pwf1f[P1:P0, :], A1[:, P0:C],
                                 wp_sb[0:P1, 192:384],
                                 start=False, stop=True)
                nc.scalar.copy(wf_sb[:, 0:192], pwf0[:])
                nc.scalar.copy(wf_sb[P1:P0, 192:384], pwf1f[P1:P0, :])

                # ==== final sweep: out = WfT-contraction @ v_dw ====
                for i in range(HW // SUB):
                    if i % 2 == 0:
                        po0 = psH.tile([P0, SUB], f32, tag="tap0")
                        po1f = psH.tile([P0, SUB], f32, tag="tap1")
                    else:
                        po0 = psA.tile([P0, SUB], f32, tag="pw")
                        po1f = psA.tile([P0, SUB], f32, tag="pw")
                    po1 = po1f[0:P1, :]
                    r0 = vdw_sb[:, i * SUB:(i + 1) * SUB]
                    r1 = vdw_sb[P1:P0, HW + i * SUB:HW + (i + 1) * SUB]
                    if "final" not in _abl:
                        nc.tensor.matmul(po0[:], wf_sb[:, 0:P0], r0,
                                         start=True, stop=False)
                        nc.tensor.matmul(po0[:], wf_sb[P1:P0, 192:320], r1,
                                         start=False, stop=True,
                                         tile_position=(P1, 0))
                        nc.tensor.matmul(po1, wf_sb[:, P0:192], r0,
                                         start=True, stop=False)
                        nc.tensor.matmul(po1, wf_sb[P1:P0, 320:384], r1,
                                         start=False, stop=True,
                                         tile_position=(P1, 0))
                    else:
                        nc.vector.memset(po0[:, 0:1], 0.0)
                        nc.vector.memset(po1[:, 0:1], 0.0)
                    ost0 = wkp.tile([P0, SUB], f16, tag="ost0")
                    ost1 = wkp.tile([P1, SUB], f16, tag="ost1")
                    nc.scalar.copy(ost0[:], po0[:])
                    nc.vector.tensor_copy(ost1[:], po1)
                    nc.sync.dma_start(out=out[0:P0, i * SUB:(i + 1) * SUB],
                                      in_=ost0[:])
                    nc.sync.dma_start(out=out[P0:C, i * SUB:(i + 1) * SUB],
                                      in_=ost1[:])

            # ========== schedule ==========
            nc.sync.dma_start(out=wqv_sb[:, 0:640], in_=wqv[:, 0:640])
            nc.sync.dma_start(out=wqv_sb[0:P1, 640:768], in_=wqv[0:P1, 640:768])
            for b in range(NB):
                pw_stage(b)
                if b == 0:
                    load_rest_consts()
                if b >= 1:
                    qtap_stage(b - 1)
                if b >= 2:
                    gramm_stage(b - 2)
                    if b - 2 < DEFER:
                        vtap_stage(b - 2)
                if b >= 1:
                    trans_stage(b - 1)
            qtap_stage(NB - 1)
            gramm_stage(NB - 2)
            trans_stage(NB - 1)
            vtap_stage(DEFER, pe_all=True)
            gramm_stage(NB - 1)
            middle()
            for b in range(DEFER + 1, NB):
                vtap_stage(b, pe_all=True)
            wf_and_final()

    nc.compile()
    return nc


def _host_inputs(x, w_qkv, w_dw, w_proj, temperature):
    """Per-core input maps (host-side precompute of all weight transforms)."""
    f = np.float32
    W_q = w_qkv[0:C].astype(f)           # (192,192) out x in
    W_v = w_qkv[2 * C:3 * C].astype(f)
    wq_d = w_dw[0:C, 0].reshape(C, 9).astype(f)        # (192,9) taps (di,dj)
    wv_d = w_dw[2 * C:3 * C, 0].reshape(C, 9).astype(f)

    WqT = W_q.T.astype(f)                # (in, out)
    WvT = W_v.T.astype(f)
    wqv = np.zeros((P0, 768), f)
    wqv[:, 0:128] = WqT[0:P0, 0:128]
    wqv[0:P1, 128:256] = WqT[P0:C, 0:128]
    wqv[:, 256:384] = WvT[0:P0, 0:128]
    wqv[0:P1, 384:512] = WvT[P0:C, 0:128]
    # chunk1 pointwise output order: [v1 (parts 0:64); q1 (parts 64:128)]
    wqv[:, 512:576] = WvT[0:P0, 128:192]
    wqv[:, 576:640] = WqT[0:P0, 128:192]
    wqv[0:P1, 640:704] = WvT[P0:C, 128:192]
    wqv[0:P1, 704:768] = WqT[P0:C, 128:192]

    def pack_diag(wd, lo, n):
        outm = np.zeros((n, 9 * n), f)
        for t in range(9):
            np.fill_diagonal(outm[:, t * n:(t + 1) * n], wd[lo:lo + n, t])
        return outm

    wp_pack = np.zeros((P0, 384), f)
    WpT = w_proj.T.astype(f)
    wp_pack[:, 0:192] = WpT[0:P0]
    wp_pack[0:P1, 192:384] = WpT[P0:C]

    # chunk1 paired/single tap diagonals.
    # v1 buffer: orig at parts 0:64 (window dj=1 -> tap (di,1)),
    #            dup  at parts 64:128 (window dj=1 -> tap (di,0)).
    dv1p_m = np.zeros((P0, 3 * P1), f)
    dv1s_m = np.zeros((P1, 3 * P1), f)
    # q1 buffer: dup at parts 0:64 (-> tap (di,0)), orig at 64:128 (-> (di,1))
    dq1p_m = np.zeros((P0, 3 * P1), f)
    dq1s_m = np.zeros((P0, 3 * P1), f)
    for di in range(3):
        sl = slice(di * P1, (di + 1) * P1)
        np.fill_diagonal(dv1p_m[0:P1, sl], wv_d[P0:C, 3 * di + 1])
        np.fill_diagonal(dv1p_m[P1:P0, sl], wv_d[P0:C, 3 * di + 0])
        np.fill_diagonal(dv1s_m[:, sl], wv_d[P0:C, 3 * di + 2])
        np.fill_diagonal(dq1p_m[0:P1, sl], wq_d[P0:C, 3 * di + 0])
        np.fill_diagonal(dq1p_m[P1:P0, sl], wq_d[P0:C, 3 * di + 1])
        np.fill_diagonal(dq1s_m[P1:P0, sl], wq_d[P0:C, 3 * di + 2])

    tqv = np.sqrt(np.repeat(temperature.reshape(HEADS).astype(f),
                            CHD)).reshape(C, 1)
    eye = np.eye(P0, dtype=np.float16)

    heads0 = np.arange(P0) // CHD
    heads1 = (P0 + np.arange(P1)) // CHD
    headc = np.arange(C) // CHD
    bm0_m = (headc[None, :] == heads0[:, None]).astype(f)
    bm1_m = (headc[None, :] == heads1[:, None]).astype(f)
    hm0_m = (np.arange(HEADS)[None, :] == heads0[:, None]).astype(f)
    hm1_m = (np.arange(HEADS)[None, :] == heads1[:, None]).astype(f)

    shared = {
        "wqv": wqv, "wp": wp_pack,
        "dq": pack_diag(wq_d, 0, P0),
        "dv": pack_diag(wv_d, 0, P0),
        "dvw": np.ascontiguousarray(wv_d[0:P0]).astype(f),
        "dv1p": dv1p_m, "dv1s": dv1s_m, "dq1p": dq1p_m, "dq1s": dq1s_m,
        "tq": tqv, "eye": eye,
        "bm0": bm0_m, "bm1": bm1_m, "hm0": hm0_m, "hm1": hm1_m,
    }
    h = np.float16
    for k in ("wqv", "dq", "dv", "dv1p", "dv1s", "dq1p", "dq1s"):
        shared[k] = shared[k].astype(h)
    maps = []
    for b in range(8):
        m = dict(shared)
        m["xb"] = np.ascontiguousarray(x[b].reshape(C, HW).astype(h))
        maps.append(m)
    return maps


def kernel(x, w_qkv, w_dw, w_proj, temperature, _trace=False, _iters=1):
    from concourse.bass_utils import run_bass_kernel_spmd
    if _iters not in _BUILT:
        _BUILT[_iters] = _build(_iters)
    nc = _BUILT[_iters]
    in_maps = _host_inputs(
        np.asarray(x), np.asarray(w_qkv), np.asarray(w_dw),
        np.asarray(w_proj), np.asarray(temperature))
    res = run_bass_kernel_spmd(nc, in_maps, list(range(8)), trace=_trace)
    outs = [res.results[i]["out"].reshape(C, H, W) for i in range(8)]
    y = np.stack(outs, axis=0).astype(np.float32)
    kernel.last_result = res
    return y
